# revision 1
# baseline (speedup 1.0000x reference)
"""2-layer GAT forward on 8 Trainium2 NeuronCores.

Strategy: target-node sharding. Nodes are degree-sorted and dealt round-robin
to 8 cores in groups of 128 (all cores share per-group padded degree D_g).
Each core fetches per-edge source rows with the MoE dma_gather primitive from
a replicated node table (signed int16 indices offset from a midpoint base row
cover the whole table), does the segment softmax as a dense reduction over
the padded degree axis, and produces output rows for its own nodes. The only
cross-core exchange is an AllGather of the small layer-2 node table [N, 8].
"""

import math
import numpy as np
import ml_dtypes

import concourse.bass as bass
import concourse.mybir as mybir
from concourse import bacc
from concourse.tile import TileContext
from concourse.bass_utils import run_bass_kernel_spmd
from concourse.masks import make_identity

BF16 = ml_dtypes.bfloat16

NC = 8
P = 128
FIN = 128   # layer-1 input features
HF = 64     # H*F layer 1
H1 = 8
F1 = 8
C2 = 7      # layer-2 out features
R1 = 128    # tab1 row elems (f32) = 512B
R2 = 64     # tab2 row elems (f32) = 256B
MASKVAL = -150.0

_CACHE = {}
DEBUG_DUMP = False


# --------------------------------------------------------------------------
# device kernel builder
# --------------------------------------------------------------------------

def _build(npad, nslice, g_cnt, dg, total_slots):
    DT = mybir.dt
    fp32 = DT.float32
    base = 32768 if npad > 32768 else 0
    nc = bacc.Bacc("TRN2", target_bir_lowering=False, debug=False, num_devices=NC)

    xT = nc.dram_tensor("xT", [P, npad], fp32, kind="ExternalInput")
    xownT = nc.dram_tensor("xownT", [P, nslice], fp32, kind="ExternalInput")
    # wrapped + core-replicated int16 gather indices, shared by both layers
    idx = nc.dram_tensor("idx", [P, total_slots // 16], DT.int16, kind="ExternalInput")
    srcmask = nc.dram_tensor("srcmask", [npad], fp32, kind="ExternalInput")
    srcmaskown = nc.dram_tensor("srcmaskown", [nslice], fp32, kind="ExternalInput")
    wcat1 = nc.dram_tensor("wcat1", [P, 80], fp32, kind="ExternalInput")
    w2cat = nc.dram_tensor("w2cat", [HF, 16], fp32, kind="ExternalInput")
    b1d = nc.dram_tensor("b1d", [HF], fp32, kind="ExternalInput")
    b2d = nc.dram_tensor("b2d", [C2], fp32, kind="ExternalInput")
    out = nc.dram_tensor("out", [nslice, C2], fp32, kind="ExternalOutput")

    tab1 = nc.dram_tensor("tab1", [npad, R1], fp32)
    tab2in = nc.dram_tensor("tab2in", [nslice, 8], fp32)
    tab2c = nc.dram_tensor("tab2c", [npad, 8], fp32, addr_space="Shared")
    tab2f = nc.dram_tensor("tab2f", [npad, R2], fp32)

    dbg = {}
    if DEBUG_DUMP:
        dbg["tab1o"] = nc.dram_tensor("tab1o", [npad, R1], fp32, kind="ExternalOutput")
        dbg["tab2o"] = nc.dram_tensor("tab2o", [npad, R2], fp32, kind="ExternalOutput")
        dbg["g1o"] = nc.dram_tensor("g1o", [P, dg[0] * R1], fp32, kind="ExternalOutput")
        dbg["h0o"] = nc.dram_tensor("h0o", [P, HF], fp32, kind="ExternalOutput")

    n_tiles = npad // P
    chunk = 4
    n_chunks = n_tiles // chunk
    offs = np.concatenate([[0], np.cumsum([P * d for d in dg])]).astype(int)

    with TileContext(nc) as tc:
        with (
            tc.tile_pool(name="persist", bufs=1) as pp,
            tc.tile_pool(name="pA_x", bufs=3) as pa_x,
            tc.tile_pool(name="pA_tab", bufs=4) as pa_tab,
            tc.tile_pool(name="pA_ps", bufs=2, space="PSUM") as pa_ps,
            tc.tile_pool(name="pB_g", bufs=3) as pb_g,
            tc.tile_pool(name="pIdx", bufs=3) as pidx,
            tc.tile_pool(name="pB_sc", bufs=2) as pb_sc,
            tc.tile_pool(name="pB_msg", bufs=1) as pb_msg,
            tc.tile_pool(name="pB_sm", bufs=3) as pb_sm,
            tc.tile_pool(name="pD_ps", bufs=2, space="PSUM") as pd_ps,
            tc.tile_pool(name="pE_g", bufs=3) as pe_g,
            tc.tile_pool(name="pM2", bufs=1) as pm2,
            tc.tile_pool(name="pE_sm", bufs=3) as pe_sm,
        ):
            # ---- persistent small tiles ----
            wcat1_sb = pp.tile([P, 80], fp32, tag="wcat1")
            nc.sync.dma_start(out=wcat1_sb[:], in_=wcat1[:])
            w2cat_sb = pp.tile([HF, 16], fp32, tag="w2cat")
            nc.sync.dma_start(out=w2cat_sb[:], in_=w2cat[:])
            srcmask_sb = pp.tile([P, n_tiles], fp32, tag="srcmask")
            nc.sync.dma_start(out=srcmask_sb[:], in_=srcmask.ap().rearrange("(t p) -> p t", p=P))
            srcmaskown_sb = pp.tile([P, g_cnt], fp32, tag="srcmaskown")
            nc.sync.dma_start(out=srcmaskown_sb[:], in_=srcmaskown.ap().rearrange("(g p) -> p g", p=P))
            ones_sb = pp.tile([1, P], fp32, tag="ones")
            nc.vector.memset(ones_sb[:], 1.0)
            b1_sb = pp.tile([1, HF], fp32, tag="b1sb")
            nc.sync.dma_start(out=b1_sb[:], in_=b1d.ap().rearrange("(o c) -> o c", o=1))
            b2m_sb = pp.tile([1, 8], fp32, tag="b2msb")
            nc.vector.memset(b2m_sb[:], 0.0)
            nc.sync.dma_start(out=b2m_sb[:, 0:C2], in_=b2d.ap().rearrange("(o c) -> o c", o=1))
            ident_sb = pp.tile([P, P], fp32, tag="ident")
            make_identity(nc, ident_sb[:])

            b1bc_ps = pa_ps.tile([P, HF], fp32, tag="projps")
            nc.tensor.matmul(out=b1bc_ps[:], lhsT=ones_sb[:], rhs=b1_sb[:], start=True, stop=True)
            b1bc_sb = pp.tile([P, HF], fp32, tag="b1bc")
            nc.vector.tensor_copy(out=b1bc_sb[:], in_=b1bc_ps[:])
            b2bc_ps = pa_ps.tile([P, 8], fp32, tag="projps")
            nc.tensor.matmul(out=b2bc_ps[:], lhsT=ones_sb[:], rhs=b2m_sb[:], start=True, stop=True)
            b2bc_sb = pp.tile([P, 8], fp32, tag="b2bc")
            nc.vector.tensor_copy(out=b2bc_sb[:], in_=b2bc_ps[:])

            tab2slice_sb = pp.tile([P, g_cnt * 8], fp32, tag="tab2slice")
            strg2_sb = pp.tile([P, g_cnt], fp32, tag="strg2")
            strgown_sb = pp.tile([P, g_cnt * 8], fp32, tag="strgown")

            # ---- phase A: node table (replicated), 4 tiles batched per op ----
            for cidx in range(n_chunks):
                xc = pa_x.tile([P, chunk * P], fp32, tag="xchunk")
                nc.sync.dma_start(
                    out=xc[:], in_=xT[:, cidx * chunk * P:(cidx + 1) * chunk * P])
                psC = pa_ps.tile([P, chunk * 80], fp32, tag="projps")
                for j in range(chunk):
                    nc.tensor.matmul(out=psC[:, j * 80:(j + 1) * 80],
                                     lhsT=xc[:, j * P:(j + 1) * P],
                                     rhs=wcat1_sb[:], start=True, stop=True)
                tabt = pa_tab.tile([P, chunk * 80], fp32, tag="tabt")
                psv = psC[:].rearrange("p (j c) -> p j c", c=80)
                tbv = tabt[:].rearrange("p (j c) -> p j c", c=80)
                nc.vector.tensor_add(
                    out=tbv[:, :, 0:HF], in0=psv[:, :, 0:HF],
                    in1=b1bc_sb[:].rearrange("p (j c) -> p j c", j=1
                                             ).to_broadcast([P, chunk, HF]))
                nc.vector.tensor_add(
                    out=tbv[:, :, HF:72], in0=psv[:, :, HF:72],
                    in1=srcmask_sb[:, cidx * chunk:(cidx + 1) * chunk]
                    .rearrange("p (j c) -> p j c", c=1).to_broadcast([P, chunk, 8]))
                nc.scalar.copy(out=tbv[:, :, 72:80], in_=psv[:, :, 72:80])
                nc.sync.dma_start(
                    out=tab1.ap()[cidx * chunk * P:(cidx + 1) * chunk * P, 0:80]
                    .rearrange("(j p) c -> p j c", p=P),
                    in_=tbv)

            # own-node s_trg1 via small matmuls on xownT
            for g in range(g_cnt):
                xog = pa_x.tile([P, P], fp32, tag="xog")
                nc.sync.dma_start(out=xog[:], in_=xownT[:, g * P:(g + 1) * P])
                pso = pa_ps.tile([P, 80], fp32, tag="projps")
                nc.tensor.matmul(out=pso[:, 0:8], lhsT=xog[:],
                                 rhs=wcat1_sb[:, 72:80], start=True, stop=True)
                nc.vector.tensor_copy(out=strgown_sb[:, g * 8:(g + 1) * 8], in_=pso[:, 0:8])

            # ---- phases B and D, per group ----
            for g in range(g_cnt):
                D = dg[g]
                L = P * D
                idxg = pidx.tile([P, (offs[g + 1] - offs[g]) // 16], DT.int16, tag="idxg")
                nc.sync.dma_start(out=idxg[:], in_=idx[:, offs[g] // 16:offs[g + 1] // 16])
                g1 = pb_g.tile([P, D * R1], fp32, tag="g1")
                nc.gpsimd.dma_gather(
                    out_ap=g1[:].rearrange("p (d c) -> p d c", c=R1),
                    in_ap=tab1[base:, :],
                    idxs_ap=idxg[:],
                    num_idxs=L, num_idxs_reg=L, elem_size=R1,
                    single_packet=False)
                g1v = g1[:].rearrange("p (d c) -> p d c", c=R1)
                if DEBUG_DUMP and g == 0:
                    nc.sync.dma_start(out=dbg["g1o"][:], in_=g1[:])

                sc = pb_sc.tile([P, D * 8], fp32, tag="scores")
                scv = sc[:].rearrange("p (d h) -> p d h", h=H1)
                strg_g = strgown_sb[:, g * 8:(g + 1) * 8]
                nc.vector.tensor_add(
                    out=scv, in0=g1v[:, :, HF:72],
                    in1=strg_g.rearrange("p (d h) -> p d h", d=1).to_broadcast([P, D, H1]))
                nc.vector.scalar_tensor_tensor(
                    out=sc[:], in0=sc[:], scalar=0.2, in1=sc[:],
                    op0=mybir.AluOpType.mult, op1=mybir.AluOpType.max)
                nc.scalar.activation(out=sc[:], in_=sc[:],
                                     func=mybir.ActivationFunctionType.Exp)
                ssum = pb_sm.tile([P, 8], fp32, tag="ssum")
                nc.vector.tensor_reduce(
                    out=ssum[:], in_=sc[:].rearrange("p (d h) -> p h d", h=H1),
                    axis=mybir.AxisListType.X, op=mybir.AluOpType.add)
                rinv = pb_sm.tile([P, 8], fp32, tag="rinv")
                nc.vector.reciprocal(out=rinv[:], in_=ssum[:])

                msg = pb_msg.tile([P, D * HF], fp32, tag="msg")
                nc.vector.tensor_mul(
                    out=msg[:].rearrange("p (d h f) -> p d h f", h=H1, f=F1),
                    in0=g1v[:, :, 0:HF].rearrange("p d (h f) -> p d h f", f=F1),
                    in1=sc[:].rearrange("p (d h f) -> p d h f", h=H1, f=1
                                        ).to_broadcast([P, D, H1, F1]))
                out1 = pb_sm.tile([P, HF], fp32, tag="out1")
                nc.vector.tensor_reduce(
                    out=out1[:], in_=msg[:].rearrange("p (d h f) -> p h f d", h=H1, f=F1),
                    axis=mybir.AxisListType.X, op=mybir.AluOpType.add)
                nc.vector.tensor_mul(
                    out=out1[:].rearrange("p (h f) -> p h f", h=H1), 
                    in0=out1[:].rearrange("p (h f) -> p h f", h=H1),
                    in1=rinv[:].rearrange("p (h f) -> p h f", f=1
                                          ).to_broadcast([P, H1, F1]))

                # ELU -> h (b1 folded into the table)
                uu = pb_sm.tile([P, HF], fp32, tag="uu")
                nc.vector.tensor_scalar_min(out=uu[:], in0=out1[:], scalar1=0.0)
                nc.scalar.activation(out=uu[:], in_=uu[:],
                                     func=mybir.ActivationFunctionType.Exp)
                nc.vector.tensor_scalar_max(out=out1[:], in0=out1[:], scalar1=0.0)
                hh = pb_sm.tile([P, HF], fp32, tag="hh")
                nc.vector.tensor_add(out=hh[:], in0=uu[:], in1=out1[:])
                nc.scalar.activation(out=hh[:], in_=hh[:],
                                     func=mybir.ActivationFunctionType.Copy, bias=-1.0)
                if DEBUG_DUMP and g == 0:
                    nc.sync.dma_start(out=dbg["h0o"][:], in_=hh[:])

                # ---- phase D ----
                psT = pd_ps.tile([HF, P], fp32, tag="psT")
                nc.tensor.transpose(out=psT[:], in_=hh[:], identity=ident_sb[:])
                hT = pb_sm.tile([HF, P], fp32, tag="hT")
                nc.vector.tensor_copy(out=hT[:], in_=psT[:])
                ps2 = pd_ps.tile([P, 9], fp32, tag="ps2")
                nc.tensor.matmul(out=ps2[:], lhsT=hT[:], rhs=w2cat_sb[:, 0:9],
                                 start=True, stop=True)
                t2s = tab2slice_sb[:, g * 8:(g + 1) * 8]
                nc.vector.tensor_add(out=t2s, in0=ps2[:, 0:8], in1=b2bc_sb[:])
                nc.vector.tensor_add(
                    out=tab2slice_sb[:, g * 8 + 7:g * 8 + 8],
                    in0=tab2slice_sb[:, g * 8 + 7:g * 8 + 8],
                    in1=srcmaskown_sb[:, g:g + 1])
                nc.vector.tensor_copy(out=strg2_sb[:, g:g + 1], in_=ps2[:, 8:9])

            # ---- phase C: exchange + expand layer-2 table ----
            nc.sync.dma_start(
                out=tab2in.ap().rearrange("(g p) c -> p g c", p=P),
                in_=tab2slice_sb[:].rearrange("p (g c) -> p g c", c=8))
            nc.gpsimd.collective_compute(
                "AllGather",
                mybir.AluOpType.bypass,
                ins=[tab2in[:]],
                outs=[tab2c[:]],
                replica_groups=[list(range(NC))],
            )
            nc.sync.dma_start(out=tab2f.ap()[:, 0:8], in_=tab2c[:])
            if DEBUG_DUMP:
                nc.sync.dma_start(out=dbg["tab1o"][:], in_=tab1[:])
                nc.sync.dma_start(out=dbg["tab2o"][:], in_=tab2f[:])

            # ---- phase E: layer 2 per group ----
            for g in range(g_cnt):
                D = dg[g]
                L = P * D
                idxg = pidx.tile([P, (offs[g + 1] - offs[g]) // 16], DT.int16, tag="idxg")
                nc.sync.dma_start(out=idxg[:], in_=idx[:, offs[g] // 16:offs[g + 1] // 16])
                g2 = pe_g.tile([P, D * R2], fp32, tag="g2")
                nc.gpsimd.dma_gather(
                    out_ap=g2[:].rearrange("p (d c) -> p d c", c=R2),
                    in_ap=tab2f[base:, :],
                    idxs_ap=idxg[:],
                    num_idxs=L, num_idxs_reg=L, elem_size=R2,
                    single_packet=False)
                g2v = g2[:].rearrange("p (d c) -> p d c", c=R2)

                sc2 = pe_sm.tile([P, D], fp32, tag="sc2")
                nc.vector.tensor_scalar_add(
                    out=sc2[:],
                    in0=g2v[:, :, 7:8].rearrange("p d c -> p (d c)"),
                    scalar1=strg2_sb[:, g:g + 1])
                nc.vector.scalar_tensor_tensor(
                    out=sc2[:], in0=sc2[:], scalar=0.2, in1=sc2[:],
                    op0=mybir.AluOpType.mult, op1=mybir.AluOpType.max)
                ssum2 = pe_sm.tile([P, 1], fp32, tag="ssum2")
                nc.scalar.activation(out=sc2[:], in_=sc2[:],
                                     func=mybir.ActivationFunctionType.Exp,
                                     accum_out=ssum2[:])
                rinv2 = pe_sm.tile([P, 1], fp32, tag="rinv2")
                nc.vector.reciprocal(out=rinv2[:], in_=ssum2[:])

                m2 = pm2.tile([P, D * 8], fp32, tag="m2")
                nc.vector.tensor_mul(
                    out=m2[:].rearrange("p (d c) -> p d c", c=8),
                    in0=g2v[:, :, 0:8],
                    in1=sc2[:].rearrange("p (d c) -> p d c", c=1).to_broadcast([P, D, 8]))
                o2 = pe_sm.tile([P, 8], fp32, tag="o2")
                nc.vector.tensor_reduce(
                    out=o2[:], in_=m2[:].rearrange("p (d c) -> p c d", c=8),
                    axis=mybir.AxisListType.X, op=mybir.AluOpType.add)
                nc.vector.tensor_scalar_mul(out=o2[:], in0=o2[:], scalar1=rinv2[:])

                negmax = pe_sm.tile([P, 1], fp32, tag="negmax")
                nc.vector.tensor_reduce(
                    out=negmax[:], in_=o2[:, 0:C2], axis=mybir.AxisListType.X,
                    op=mybir.AluOpType.max, negate=True)
                sum7 = pe_sm.tile([P, 1], fp32, tag="sum7")
                e7 = pe_sm.tile([P, C2], fp32, tag="e7")
                nc.scalar.activation(out=e7[:], in_=o2[:, 0:C2],
                                     func=mybir.ActivationFunctionType.Exp,
                                     bias=negmax[:], accum_out=sum7[:])
                r7 = pe_sm.tile([P, 1], fp32, tag="r7")
                nc.vector.reciprocal(out=r7[:], in_=sum7[:])
                res = pe_sm.tile([P, C2], fp32, tag="res")
                nc.vector.tensor_scalar_mul(out=res[:], in0=e7[:], scalar1=r7[:])
                nc.sync.dma_start(out=out[g * P:(g + 1) * P, :], in_=res[:])

    nc.compile()
    return nc


# --------------------------------------------------------------------------
# host side
# --------------------------------------------------------------------------

def _preprocess(x, edge_index):
    src = np.asarray(edge_index[0], np.int64)
    trg = np.asarray(edge_index[1], np.int64)
    n = x.shape[0]
    e = src.shape[0]

    deg = np.bincount(trg, minlength=n)
    order = np.argsort(-deg, kind="stable")          # rank -> node
    g_cnt = math.ceil(n / (P * NC))
    if g_cnt * P * NC == n:
        g_cnt += 1  # ensure pad rows exist (dummy index must be a pad row)
    npad = g_cnt * P * NC
    nslice = g_cnt * P

    ranks = np.empty(n, np.int64)
    ranks[order] = np.arange(n)
    core_of = ranks % NC
    grp_of = ranks // (P * NC)
    slot_of = (ranks // NC) % P
    perm = core_of * nslice + grp_of * P + slot_of   # node -> perm position

    # per-group padded degree, shared across cores; make sure the LAST list
    # slot of each group is padding (trailing negative-idx trim on HW)
    dg = []
    for g in range(g_cnt):
        w = order[P * NC * g: P * NC * (g + 1)]
        if len(w) == 0:
            dg.append(1)
            continue
        degs = deg[w]  # already descending
        dmax = max(int(degs.max()), 1)
        if len(degs) <= 1016 or int(degs[1016:].max()) == dmax:
            dmax += 1
        dg.append(dmax)
    offs = np.concatenate([[0], np.cumsum([P * d for d in dg])]).astype(np.int64)
    total_slots = int(offs[-1])

    dummy = npad - 1  # a pad position
    base = 32768 if npad > 32768 else 0

    tp = perm[trg]
    eorder = np.argsort(tp, kind="stable")
    tps = tp[eorder]
    counts = np.bincount(tps, minlength=npad)
    starts = np.concatenate([[0], np.cumsum(counts)[:-1]])
    d_of = np.arange(e) - starts[tps]

    c_of = tps // nslice
    r_local = tps % nslice
    g_of = r_local // P
    p_of = r_local % P
    pos = offs[g_of] + d_of * P + p_of               # k = d*128 + p within group

    idx_flat = np.full((NC, total_slots), dummy - base, np.int16)
    idx_flat[c_of, pos] = (perm[src[eorder]] - base).astype(np.int16)

    # wrap for dma_gather: element k -> (partition k%16, col k//16), per group,
    # then replicate the 16-partition block to all 128 partitions
    idx_wrapped = np.empty((NC, P, total_slots // 16), np.int16)
    for g in range(g_cnt):
        lo, hi = int(offs[g]), int(offs[g + 1])
        blk = idx_flat[:, lo:hi].reshape(NC, (hi - lo) // 16, 16)  # [c, col, p16]
        wr = np.swapaxes(blk, 1, 2)                                # [c, p16, col]
        idx_wrapped[:, :, lo // 16:hi // 16] = np.tile(wr, (1, 8, 1))

    pad_mask = np.ones(npad, bool)
    pad_mask[perm] = False

    return dict(n=n, e=e, npad=npad, nslice=nslice, g_cnt=g_cnt, dg=dg,
                total_slots=total_slots, perm=perm, idx_wrapped=idx_wrapped,
                pad_mask=pad_mask, base=base)


def _prepare(x, edge_index, W1, a_src1, a_trg1, b1, W2, a_src2, a_trg2, b2):
    x = np.asarray(x, np.float32)
    W1 = np.asarray(W1, np.float32)
    a_src1 = np.asarray(a_src1, np.float32)
    a_trg1 = np.asarray(a_trg1, np.float32)
    b1 = np.asarray(b1, np.float32)
    W2 = np.asarray(W2, np.float32)
    a_src2 = np.asarray(a_src2, np.float32)
    a_trg2 = np.asarray(a_trg2, np.float32)
    b2 = np.asarray(b2, np.float32)

    meta = _preprocess(x, edge_index)
    npad, nslice, g_cnt = meta["npad"], meta["nslice"], meta["g_cnt"]
    perm = meta["perm"]

    xp = np.zeros((npad, FIN), np.float32)
    xp[perm] = x
    xT = np.ascontiguousarray(xp.T)

    srcmask = np.where(meta["pad_mask"], np.float32(MASKVAL), np.float32(0.0))
    srcmaskown = srcmask[:nslice].copy()  # same local pad pattern on every core

    Wt = W1.T  # [128, 64], col = h*F + j
    w3 = W1.reshape(H1, F1, FIN)
    Asrc = np.einsum("hjf,hj->fh", w3, a_src1[0])
    Atrg = np.einsum("hjf,hj->fh", w3, a_trg1[0])
    wcat1 = np.ascontiguousarray(np.concatenate([Wt, Asrc, Atrg], axis=1), np.float32)

    w2cat = np.zeros((HF, 16), np.float32)
    w2cat[:, 0:C2] = W2.T
    w2cat[:, C2] = W2.T @ a_src2[0, 0]
    w2cat[:, C2 + 1] = W2.T @ a_trg2[0, 0]

    key = (npad, g_cnt, tuple(meta["dg"]))
    if key not in _CACHE:
        _CACHE[key] = _build(npad, nslice, g_cnt, meta["dg"], meta["total_slots"])
    nc = _CACHE[key]

    in_maps = []
    for c in range(NC):
        xownT = np.ascontiguousarray(xp[c * nslice:(c + 1) * nslice].T)
        in_maps.append({
            "xT": xT,
            "xownT": xownT,
            "idx": np.ascontiguousarray(meta["idx_wrapped"][c]),
            "srcmask": srcmask,
            "srcmaskown": srcmaskown,
            "wcat1": wcat1,
            "w2cat": w2cat,
            "b1d": b1,
            "b2d": b2,
        })
    return nc, in_maps, meta


def kernel(x, edge_index, W1, a_src1, a_trg1, b1, W2, a_src2, a_trg2, b2):
    nc, in_maps, meta = _prepare(x, edge_index, W1, a_src1, a_trg1, b1,
                                 W2, a_src2, a_trg2, b2)
    res = run_bass_kernel_spmd(nc, in_maps, core_ids=list(range(NC)))
    full = np.concatenate([res.results[c]["out"] for c in range(NC)], axis=0)
    return full[meta["perm"]].astype(np.float32)



# revision 2
# speedup vs baseline: 1.6942x; 1.6942x over previous
"""2-layer GAT forward on 8 Trainium2 NeuronCores.

Strategy: target-node sharding (degree-sorted round-robin groups of 128).
v2: gathers ride 4 SWDGE queues round-robin (4 DSP desc-gen pairs in
parallel), descriptor rings enlarged so gen overlaps drain, tables packed
to cut gather HBM bytes: layer-1 rows are 160B (proj64 bf16 + s_src8 f32),
layer-2 rows are 32B (8 f32), both fetched with a raw gather instruction
(the 256B-elem floor in the bass helper is a transpose-only restriction).
"""

import math
import numpy as np
import ml_dtypes

import concourse.bass as bass
import concourse.mybir as mybir
from concourse import bacc
from concourse.tile import TileContext
from concourse.bass_utils import run_bass_kernel_spmd
from concourse.masks import make_identity
from concourse import ap_utils

BF16 = ml_dtypes.bfloat16

NC = 8
P = 128
FIN = 128
HF = 64
H1 = 8
F1 = 8
C2 = 7
R1 = 128    # tab1 row stride in bf16 elems (256B)
E1 = 80     # tab1 gathered elems (bf16): 64 proj + 16 (=8 f32 s_src)
R2 = 64     # tab2 row stride in f32 elems (256B)
E2 = 8      # tab2 gathered elems (f32)
MASKVAL = -150.0
NQ = 4      # SWDGE queues

_CACHE = {}


def _exact_div(a, b):
    assert a % b == 0
    return a // b


def _gather_raw(eng, out_ap, in_ap, idxs_ap, num_idxs, elem_size, elem_step,
                queue_num):
    """nc.gpsimd.dma_gather minus the %256 elem assert (transpose-only
    ucode restriction; non-transpose descriptors take arbitrary sizes)."""
    eng._assert_queue_num(queue_num)
    assert idxs_ap.dtype == mybir.dt.int16
    assert in_ap.dtype == out_ap.dtype
    assert in_ap.space == bass.MemorySpace.DRAM
    assert idxs_ap.space == bass.MemorySpace.SBUF
    assert out_ap.space == bass.MemorySpace.SBUF
    assert ap_utils.ap_is_contiguous(out_ap.ap[1:])
    assert ap_utils.ap_is_contiguous(idxs_ap.ap[1:])
    assert in_ap.ap[-1][1] == out_ap.ap[-1][1] == elem_size
    assert out_ap.ap[0][1] * out_ap.ap[1][1] == ((num_idxs + 127) // 128) * 128
    assert in_ap.ap[0][0] == elem_step
    stride_bytes = elem_step * mybir.dt.size(in_ap.dtype)
    stride_bytes_256 = _exact_div(stride_bytes, 256)
    assert stride_bytes_256 < 256
    _in_ap = eng.lower_ap_dma(in_ap, for_custom_bir_dma=True)
    _idxs_ap = eng.lower_ap(idxs_ap)
    _out_ap = eng.lower_ap(out_ap)
    return eng.add_instruction(
        mybir.InstDMAGatherAnt(
            name=eng.bass.get_next_instruction_name(),
            ins=[*_in_ap, _idxs_ap,
                 eng.lower_val_access(eng.to_reg(num_idxs))],
            outs=[_out_ap],
            transpose=False,
            num_idxs=num_idxs,
            elem_size=elem_size,
            stride_bytes_256=stride_bytes_256,
            gen_mode=0,
            single_packet=False,
            queue_num=queue_num,
            sbuf_tokens_per_rank=0,
            sbuf_free_dim_per_rank=0,
            sbuf_free_dim_pad_per_rank=0,
            sbuf_byte_offset=0,
        )
    )


# --------------------------------------------------------------------------
# device kernel builder
# --------------------------------------------------------------------------

def _build(npad, nslice, g_cnt, dg, total_slots):
    DT = mybir.dt
    fp32 = DT.float32
    bf16 = DT.bfloat16
    base = 32768 if npad > 32768 else 0
    nc = bacc.Bacc("TRN2", target_bir_lowering=False, debug=False,
                   num_devices=NC, num_swdge_queues=NQ,
                   dynamic_dma_scratch_size=32768)

    xT = nc.dram_tensor("xT", [P, npad], bf16, kind="ExternalInput")
    xownT = nc.dram_tensor("xownT", [P, nslice], bf16, kind="ExternalInput")
    idx = nc.dram_tensor("idx", [P, total_slots // 16], DT.int16, kind="ExternalInput")
    srcmask = nc.dram_tensor("srcmask", [npad], fp32, kind="ExternalInput")
    srcmaskown = nc.dram_tensor("srcmaskown", [nslice], fp32, kind="ExternalInput")
    wcat1 = nc.dram_tensor("wcat1", [P, 80], bf16, kind="ExternalInput")
    w2cat = nc.dram_tensor("w2cat", [HF, 16], fp32, kind="ExternalInput")
    b1d = nc.dram_tensor("b1d", [HF], fp32, kind="ExternalInput")
    b2d = nc.dram_tensor("b2d", [C2], fp32, kind="ExternalInput")
    out = nc.dram_tensor("out", [nslice, C2], fp32, kind="ExternalOutput")

    tab1 = nc.dram_tensor("tab1", [npad, R1], bf16)
    tab2in = nc.dram_tensor("tab2in", [nslice, 8], fp32)
    tab2c = nc.dram_tensor("tab2c", [npad, 8], fp32, addr_space="Shared")
    tab2f = nc.dram_tensor("tab2f", [npad, R2], fp32)

    n_tiles = npad // P
    chunk = 4
    n_chunks = n_tiles // chunk
    offs = np.concatenate([[0], np.cumsum([P * d for d in dg])]).astype(int)

    with TileContext(nc) as tc:
        with (
            tc.tile_pool(name="persist", bufs=1) as pp,
            tc.tile_pool(name="pA_x", bufs=3) as pa_x,
            tc.tile_pool(name="pA_tab", bufs=4) as pa_tab,
            tc.tile_pool(name="pA_ps", bufs=2, space="PSUM") as pa_ps,
            tc.tile_pool(name="pB_g", bufs=8) as pb_g,
            tc.tile_pool(name="pIdx", bufs=8) as pidx,
            tc.tile_pool(name="pB_sc", bufs=4) as pb_sc,
            tc.tile_pool(name="pB_msg", bufs=2) as pb_msg,
            tc.tile_pool(name="pB_sm", bufs=4) as pb_sm,
            tc.tile_pool(name="pD_ps", bufs=2, space="PSUM") as pd_ps,
            tc.tile_pool(name="pE_g", bufs=8) as pe_g,
            tc.tile_pool(name="pM2", bufs=2) as pm2,
            tc.tile_pool(name="pE_sm", bufs=4) as pe_sm,
        ):
            # ---- persistent small tiles ----
            wcat1_sb = pp.tile([P, 80], bf16, tag="wcat1")
            nc.sync.dma_start(out=wcat1_sb[:], in_=wcat1[:])
            w2cat_sb = pp.tile([HF, 16], fp32, tag="w2cat")
            nc.sync.dma_start(out=w2cat_sb[:], in_=w2cat[:])
            srcmask_sb = pp.tile([P, n_tiles], fp32, tag="srcmask")
            nc.sync.dma_start(out=srcmask_sb[:], in_=srcmask.ap().rearrange("(t p) -> p t", p=P))
            srcmaskown_sb = pp.tile([P, g_cnt], fp32, tag="srcmaskown")
            nc.sync.dma_start(out=srcmaskown_sb[:], in_=srcmaskown.ap().rearrange("(g p) -> p g", p=P))
            ones_sb = pp.tile([1, P], fp32, tag="ones")
            nc.vector.memset(ones_sb[:], 1.0)
            b1_sb = pp.tile([1, HF], fp32, tag="b1sb")
            nc.sync.dma_start(out=b1_sb[:], in_=b1d.ap().rearrange("(o c) -> o c", o=1))
            b2m_sb = pp.tile([1, 8], fp32, tag="b2msb")
            nc.vector.memset(b2m_sb[:], 0.0)
            nc.sync.dma_start(out=b2m_sb[:, 0:C2], in_=b2d.ap().rearrange("(o c) -> o c", o=1))
            ident_sb = pp.tile([P, P], fp32, tag="ident")
            make_identity(nc, ident_sb[:])

            b1bc_ps = pa_ps.tile([P, HF], fp32, tag="projps")
            nc.tensor.matmul(out=b1bc_ps[:], lhsT=ones_sb[:], rhs=b1_sb[:], start=True, stop=True)
            b1bc_sb = pp.tile([P, HF], fp32, tag="b1bc")
            nc.vector.tensor_copy(out=b1bc_sb[:], in_=b1bc_ps[:])
            b2bc_ps = pa_ps.tile([P, 8], fp32, tag="projps")
            nc.tensor.matmul(out=b2bc_ps[:], lhsT=ones_sb[:], rhs=b2m_sb[:], start=True, stop=True)
            b2bc_sb = pp.tile([P, 8], fp32, tag="b2bc")
            nc.vector.tensor_copy(out=b2bc_sb[:], in_=b2bc_ps[:])

            tab2slice_sb = pp.tile([P, g_cnt * 8], fp32, tag="tab2slice")
            strg2_sb = pp.tile([P, g_cnt], fp32, tag="strg2")
            strgown_sb = pp.tile([P, g_cnt * 8], fp32, tag="strgown")

            # ---- phase A: node table (replicated), bf16 rows of 256B:
            #      [proj64+b1 bf16 | s_src8+mask f32 | pad] ----
            for cidx in range(n_chunks):
                xc = pa_x.tile([P, chunk * P], bf16, tag="xchunk")
                nc.sync.dma_start(
                    out=xc[:], in_=xT[:, cidx * chunk * P:(cidx + 1) * chunk * P])
                psC = pa_ps.tile([P, chunk * 80], fp32, tag="projps")
                for j in range(chunk):
                    nc.tensor.matmul(out=psC[:, j * 80:(j + 1) * 80],
                                     lhsT=xc[:, j * P:(j + 1) * P],
                                     rhs=wcat1_sb[:], start=True, stop=True)
                psv = psC[:].rearrange("p (j c) -> p j c", c=80)
                nc.vector.tensor_add(
                    out=psv[:, :, 0:HF], in0=psv[:, :, 0:HF],
                    in1=b1bc_sb[:].rearrange("p (j c) -> p j c", j=1
                                             ).to_broadcast([P, chunk, HF]))
                tabt = pa_tab.tile([P, chunk * R1], bf16, tag="tabt")
                tbv = tabt[:].rearrange("p (j c) -> p j c", c=R1)
                nc.vector.tensor_copy(out=tbv[:, :, 0:HF], in_=psv[:, :, 0:HF])
                tbv32 = tabt[:].bitcast(fp32).rearrange("p (j c) -> p j c", c=R1 // 2)
                nc.vector.tensor_add(
                    out=tbv32[:, :, 32:40], in0=psv[:, :, HF:72],
                    in1=srcmask_sb[:, cidx * chunk:(cidx + 1) * chunk]
                    .rearrange("p (j c) -> p j c", c=1).to_broadcast([P, chunk, 8]))
                nc.sync.dma_start(
                    out=tab1.ap()[cidx * chunk * P:(cidx + 1) * chunk * P, :]
                    .rearrange("(j p) c -> p j c", p=P),
                    in_=tbv)

            # own-node s_trg1 via small matmuls on xownT
            for g in range(g_cnt):
                xog = pa_x.tile([P, P], bf16, tag="xog")
                nc.sync.dma_start(out=xog[:], in_=xownT[:, g * P:(g + 1) * P])
                pso = pa_ps.tile([P, 80], fp32, tag="projps")
                nc.tensor.matmul(out=pso[:, 0:8], lhsT=xog[:],
                                 rhs=wcat1_sb[:, 72:80], start=True, stop=True)
                nc.vector.tensor_copy(out=strgown_sb[:, g * 8:(g + 1) * 8], in_=pso[:, 0:8])

            # ---- phases B and D, per group ----
            for g in range(g_cnt):
                D = dg[g]
                L = P * D
                idxg = pidx.tile([P, (offs[g + 1] - offs[g]) // 16], DT.int16, tag="idxg")
                nc.sync.dma_start(out=idxg[:], in_=idx[:, offs[g] // 16:offs[g + 1] // 16])
                g1 = pb_g.tile([P, D * E1], bf16, tag="g1")
                _gather_raw(
                    nc.gpsimd,
                    out_ap=g1[:].rearrange("p (d c) -> p d c", c=E1),
                    in_ap=tab1[base:, 0:E1],
                    idxs_ap=idxg[:],
                    num_idxs=L, elem_size=E1, elem_step=R1,
                    queue_num=g % NQ)
                g1v = g1[:].rearrange("p (d c) -> p d c", c=E1)
                g1s = g1[:].bitcast(fp32).rearrange("p (d c) -> p d c", c=E1 // 2)

                sc = pb_sc.tile([P, D * 8], fp32, tag="scores")
                scv = sc[:].rearrange("p (d h) -> p d h", h=H1)
                strg_g = strgown_sb[:, g * 8:(g + 1) * 8]
                nc.vector.tensor_add(
                    out=scv, in0=g1s[:, :, 32:40],
                    in1=strg_g.rearrange("p (d h) -> p d h", d=1).to_broadcast([P, D, H1]))
                nc.vector.scalar_tensor_tensor(
                    out=sc[:], in0=sc[:], scalar=0.2, in1=sc[:],
                    op0=mybir.AluOpType.mult, op1=mybir.AluOpType.max)
                nc.scalar.activation(out=sc[:], in_=sc[:],
                                     func=mybir.ActivationFunctionType.Exp)
                ssum = pb_sm.tile([P, 8], fp32, tag="ssum")
                nc.vector.tensor_reduce(
                    out=ssum[:], in_=sc[:].rearrange("p (d h) -> p h d", h=H1),
                    axis=mybir.AxisListType.X, op=mybir.AluOpType.add)
                rinv = pb_sm.tile([P, 8], fp32, tag="rinv")
                nc.vector.reciprocal(out=rinv[:], in_=ssum[:])
                scb = pb_sc.tile([P, D * 8], bf16, tag="scb")
                nc.vector.tensor_copy(out=scb[:], in_=sc[:])

                msg = pb_msg.tile([P, D * HF], bf16, tag="msg")
                nc.vector.tensor_mul(
                    out=msg[:].rearrange("p (d h f) -> p d h f", h=H1, f=F1),
                    in0=g1v[:, :, 0:HF].rearrange("p d (h f) -> p d h f", f=F1),
                    in1=scb[:].rearrange("p (d h f) -> p d h f", h=H1, f=1
                                         ).to_broadcast([P, D, H1, F1]))
                out1 = pb_sm.tile([P, HF], fp32, tag="out1")
                nc.vector.tensor_reduce(
                    out=out1[:], in_=msg[:].rearrange("p (d h f) -> p h f d", h=H1, f=F1),
                    axis=mybir.AxisListType.X, op=mybir.AluOpType.add)
                nc.vector.tensor_mul(
                    out=out1[:].rearrange("p (h f) -> p h f", h=H1),
                    in0=out1[:].rearrange("p (h f) -> p h f", h=H1),
                    in1=rinv[:].rearrange("p (h f) -> p h f", f=1
                                          ).to_broadcast([P, H1, F1]))

                # ELU -> h (b1 folded into the table)
                uu = pb_sm.tile([P, HF], fp32, tag="uu")
                nc.vector.tensor_scalar_min(out=uu[:], in0=out1[:], scalar1=0.0)
                nc.scalar.activation(out=uu[:], in_=uu[:],
                                     func=mybir.ActivationFunctionType.Exp)
                nc.vector.tensor_scalar_max(out=out1[:], in0=out1[:], scalar1=0.0)
                hh = pb_sm.tile([P, HF], fp32, tag="hh")
                nc.vector.tensor_add(out=hh[:], in0=uu[:], in1=out1[:])
                nc.scalar.activation(out=hh[:], in_=hh[:],
                                     func=mybir.ActivationFunctionType.Copy, bias=-1.0)

                # ---- phase D ----
                psT = pd_ps.tile([HF, P], fp32, tag="psT")
                nc.tensor.transpose(out=psT[:], in_=hh[:], identity=ident_sb[:])
                hT = pb_sm.tile([HF, P], fp32, tag="hT")
                nc.vector.tensor_copy(out=hT[:], in_=psT[:])
                ps2 = pd_ps.tile([P, 9], fp32, tag="ps2")
                nc.tensor.matmul(out=ps2[:], lhsT=hT[:], rhs=w2cat_sb[:, 0:9],
                                 start=True, stop=True)
                t2s = tab2slice_sb[:, g * 8:(g + 1) * 8]
                nc.vector.tensor_add(out=t2s, in0=ps2[:, 0:8], in1=b2bc_sb[:])
                nc.vector.tensor_add(
                    out=tab2slice_sb[:, g * 8 + 7:g * 8 + 8],
                    in0=tab2slice_sb[:, g * 8 + 7:g * 8 + 8],
                    in1=srcmaskown_sb[:, g:g + 1])
                nc.vector.tensor_copy(out=strg2_sb[:, g:g + 1], in_=ps2[:, 8:9])

            # ---- phase C: exchange + expand layer-2 table ----
            nc.sync.dma_start(
                out=tab2in.ap().rearrange("(g p) c -> p g c", p=P),
                in_=tab2slice_sb[:].rearrange("p (g c) -> p g c", c=8))
            nc.gpsimd.collective_compute(
                "AllGather",
                mybir.AluOpType.bypass,
                ins=[tab2in[:]],
                outs=[tab2c[:]],
                replica_groups=[list(range(NC))],
            )
            nc.sync.dma_start(out=tab2f.ap()[:, 0:8], in_=tab2c[:])

            # ---- phase E: layer 2 per group ----
            for g in range(g_cnt):
                D = dg[g]
                L = P * D
                idxg = pidx.tile([P, (offs[g + 1] - offs[g]) // 16], DT.int16, tag="idxg")
                nc.sync.dma_start(out=idxg[:], in_=idx[:, offs[g] // 16:offs[g + 1] // 16])
                g2 = pe_g.tile([P, D * E2], fp32, tag="g2")
                _gather_raw(
                    nc.gpsimd,
                    out_ap=g2[:].rearrange("p (d c) -> p d c", c=E2),
                    in_ap=tab2f[base:, 0:E2],
                    idxs_ap=idxg[:],
                    num_idxs=L, elem_size=E2, elem_step=R2,
                    queue_num=g % NQ)
                g2v = g2[:].rearrange("p (d c) -> p d c", c=E2)

                sc2 = pe_sm.tile([P, D], fp32, tag="sc2")
                nc.vector.tensor_scalar_add(
                    out=sc2[:],
                    in0=g2v[:, :, 7:8].rearrange("p d c -> p (d c)"),
                    scalar1=strg2_sb[:, g:g + 1])
                nc.vector.scalar_tensor_tensor(
                    out=sc2[:], in0=sc2[:], scalar=0.2, in1=sc2[:],
                    op0=mybir.AluOpType.mult, op1=mybir.AluOpType.max)
                ssum2 = pe_sm.tile([P, 1], fp32, tag="ssum2")
                nc.scalar.activation(out=sc2[:], in_=sc2[:],
                                     func=mybir.ActivationFunctionType.Exp,
                                     accum_out=ssum2[:])
                rinv2 = pe_sm.tile([P, 1], fp32, tag="rinv2")
                nc.vector.reciprocal(out=rinv2[:], in_=ssum2[:])

                m2 = pm2.tile([P, D * 8], fp32, tag="m2")
                nc.vector.tensor_mul(
                    out=m2[:].rearrange("p (d c) -> p d c", c=8),
                    in0=g2v[:, :, 0:8],
                    in1=sc2[:].rearrange("p (d c) -> p d c", c=1).to_broadcast([P, D, 8]))
                o2 = pe_sm.tile([P, 8], fp32, tag="o2")
                nc.vector.tensor_reduce(
                    out=o2[:], in_=m2[:].rearrange("p (d c) -> p c d", c=8),
                    axis=mybir.AxisListType.X, op=mybir.AluOpType.add)
                nc.vector.tensor_scalar_mul(out=o2[:], in0=o2[:], scalar1=rinv2[:])

                negmax = pe_sm.tile([P, 1], fp32, tag="negmax")
                nc.vector.tensor_reduce(
                    out=negmax[:], in_=o2[:, 0:C2], axis=mybir.AxisListType.X,
                    op=mybir.AluOpType.max, negate=True)
                sum7 = pe_sm.tile([P, 1], fp32, tag="sum7")
                e7 = pe_sm.tile([P, C2], fp32, tag="e7")
                nc.scalar.activation(out=e7[:], in_=o2[:, 0:C2],
                                     func=mybir.ActivationFunctionType.Exp,
                                     bias=negmax[:], accum_out=sum7[:])
                r7 = pe_sm.tile([P, 1], fp32, tag="r7")
                nc.vector.reciprocal(out=r7[:], in_=sum7[:])
                res = pe_sm.tile([P, C2], fp32, tag="res")
                nc.vector.tensor_scalar_mul(out=res[:], in0=e7[:], scalar1=r7[:])
                nc.sync.dma_start(out=out[g * P:(g + 1) * P, :], in_=res[:])

    nc.compile()
    return nc


# --------------------------------------------------------------------------
# host side
# --------------------------------------------------------------------------

def _preprocess(x, edge_index):
    src = np.asarray(edge_index[0], np.int64)
    trg = np.asarray(edge_index[1], np.int64)
    n = x.shape[0]
    e = src.shape[0]

    deg = np.bincount(trg, minlength=n)
    order = np.argsort(-deg, kind="stable")          # rank -> node
    g_cnt = math.ceil(n / (P * NC))
    if g_cnt * P * NC == n:
        g_cnt += 1  # ensure pad rows exist (dummy index must be a pad row)
    npad = g_cnt * P * NC
    nslice = g_cnt * P

    ranks = np.empty(n, np.int64)
    ranks[order] = np.arange(n)
    core_of = ranks % NC
    grp_of = ranks // (P * NC)
    slot_of = (ranks // NC) % P
    perm = core_of * nslice + grp_of * P + slot_of   # node -> perm position

    dg = []
    for g in range(g_cnt):
        w = order[P * NC * g: P * NC * (g + 1)]
        if len(w) == 0:
            dg.append(1)
            continue
        degs = deg[w]  # already descending
        dmax = max(int(degs.max()), 1)
        if len(degs) <= 1016 or int(degs[1016:].max()) == dmax:
            dmax += 1
        dg.append(dmax)
    offs = np.concatenate([[0], np.cumsum([P * d for d in dg])]).astype(np.int64)
    total_slots = int(offs[-1])

    dummy = npad - 1  # a pad position
    base = 32768 if npad > 32768 else 0

    tp = perm[trg]
    eorder = np.argsort(tp, kind="stable")
    tps = tp[eorder]
    counts = np.bincount(tps, minlength=npad)
    starts = np.concatenate([[0], np.cumsum(counts)[:-1]])
    d_of = np.arange(e) - starts[tps]

    c_of = tps // nslice
    r_local = tps % nslice
    g_of = r_local // P
    p_of = r_local % P
    pos = offs[g_of] + d_of * P + p_of               # k = d*128 + p within group

    idx_flat = np.full((NC, total_slots), dummy - base, np.int16)
    idx_flat[c_of, pos] = (perm[src[eorder]] - base).astype(np.int16)

    idx_wrapped = np.empty((NC, P, total_slots // 16), np.int16)
    for g in range(g_cnt):
        lo, hi = int(offs[g]), int(offs[g + 1])
        blk = idx_flat[:, lo:hi].reshape(NC, (hi - lo) // 16, 16)  # [c, col, p16]
        wr = np.swapaxes(blk, 1, 2)                                # [c, p16, col]
        idx_wrapped[:, :, lo // 16:hi // 16] = np.tile(wr, (1, 8, 1))

    pad_mask = np.ones(npad, bool)
    pad_mask[perm] = False

    return dict(n=n, e=e, npad=npad, nslice=nslice, g_cnt=g_cnt, dg=dg,
                total_slots=total_slots, perm=perm, idx_wrapped=idx_wrapped,
                pad_mask=pad_mask, base=base)


def _prepare(x, edge_index, W1, a_src1, a_trg1, b1, W2, a_src2, a_trg2, b2):
    x = np.asarray(x, np.float32)
    W1 = np.asarray(W1, np.float32)
    a_src1 = np.asarray(a_src1, np.float32)
    a_trg1 = np.asarray(a_trg1, np.float32)
    b1 = np.asarray(b1, np.float32)
    W2 = np.asarray(W2, np.float32)
    a_src2 = np.asarray(a_src2, np.float32)
    a_trg2 = np.asarray(a_trg2, np.float32)
    b2 = np.asarray(b2, np.float32)

    meta = _preprocess(x, edge_index)
    npad, nslice, g_cnt = meta["npad"], meta["nslice"], meta["g_cnt"]
    perm = meta["perm"]

    xp = np.zeros((npad, FIN), np.float32)
    xp[perm] = x
    xT = np.ascontiguousarray(xp.T).astype(BF16)

    srcmask = np.where(meta["pad_mask"], np.float32(MASKVAL), np.float32(0.0))
    srcmaskown = srcmask[:nslice].copy()  # same local pad pattern on every core

    Wt = W1.T  # [128, 64], col = h*F + j
    w3 = W1.reshape(H1, F1, FIN)
    Asrc = np.einsum("hjf,hj->fh", w3, a_src1[0])
    Atrg = np.einsum("hjf,hj->fh", w3, a_trg1[0])
    wcat1 = np.concatenate([Wt, Asrc, Atrg], axis=1).astype(BF16)

    w2cat = np.zeros((HF, 16), np.float32)
    w2cat[:, 0:C2] = W2.T
    w2cat[:, C2] = W2.T @ a_src2[0, 0]
    w2cat[:, C2 + 1] = W2.T @ a_trg2[0, 0]

    key = (npad, g_cnt, tuple(meta["dg"]))
    if key not in _CACHE:
        _CACHE[key] = _build(npad, nslice, g_cnt, meta["dg"], meta["total_slots"])
    nc = _CACHE[key]

    in_maps = []
    for c in range(NC):
        xownT = np.ascontiguousarray(xp[c * nslice:(c + 1) * nslice].T).astype(BF16)
        in_maps.append({
            "xT": xT,
            "xownT": xownT,
            "idx": np.ascontiguousarray(meta["idx_wrapped"][c]),
            "srcmask": srcmask,
            "srcmaskown": srcmaskown,
            "wcat1": wcat1,
            "w2cat": w2cat,
            "b1d": b1,
            "b2d": b2,
        })
    return nc, in_maps, meta


def kernel(x, edge_index, W1, a_src1, a_trg1, b1, W2, a_src2, a_trg2, b2):
    nc, in_maps, meta = _prepare(x, edge_index, W1, a_src1, a_trg1, b1,
                                 W2, a_src2, a_trg2, b2)
    res = run_bass_kernel_spmd(nc, in_maps, core_ids=list(range(NC)))
    full = np.concatenate([res.results[c]["out"] for c in range(NC)], axis=0)
    return full[meta["perm"]].astype(np.float32)


# revision 12
# speedup vs baseline: 2.0004x; 1.1807x over previous
"""2-layer GAT forward on 8 Trainium2 NeuronCores.

Strategy: target-node sharding (degree-sorted round-robin groups of 128).
v3: gathers on 4 SWDGE queues (4 desc-gen DSP pairs in parallel, enlarged
descriptor rings so gen overlaps drain), compact table rows (160B L1 /
32B L2) via a raw gather instruction, and a vector-engine diet: leaky-relu
and exp on the scalar engine, bias/mask folded into PE matmuls, per-group
tails (ELU, reciprocals, layer-2 projection, softmax) batched across groups.
"""

import math
import numpy as np
import ml_dtypes

import concourse.bass as bass
import concourse.mybir as mybir
from concourse import bacc
from concourse.tile import TileContext
from concourse.bass_utils import run_bass_kernel_spmd
from concourse.masks import make_identity
from concourse import ap_utils

BF16 = ml_dtypes.bfloat16

NC = 8
P = 128
FIN = 128
HF = 64
H1 = 8
F1 = 8
C2 = 7
R1 = 128    # tab1 row stride in bf16 elems (256B)
E1 = 80     # tab1 gathered elems (bf16): 64 proj + 16 (=8 f32 s_src)
R2 = 64     # tab2 row stride in f32 elems (256B)
E2 = 8      # tab2 gathered elems (f32)
MASKVAL = -150.0
NQ = 4      # SWDGE queues

_CACHE = {}


def _exact_div(a, b):
    assert a % b == 0
    return a // b


def _gather_raw(eng, out_ap, in_ap, idxs_ap, num_idxs, elem_size, elem_step,
                queue_num):
    """nc.gpsimd.dma_gather minus the %256 elem assert (transpose-only
    ucode restriction; non-transpose descriptors take arbitrary sizes)."""
    eng._assert_queue_num(queue_num)
    assert idxs_ap.dtype == mybir.dt.int16
    assert in_ap.dtype == out_ap.dtype
    assert in_ap.space == bass.MemorySpace.DRAM
    assert idxs_ap.space == bass.MemorySpace.SBUF
    assert out_ap.space == bass.MemorySpace.SBUF
    assert ap_utils.ap_is_contiguous(out_ap.ap[1:])
    assert ap_utils.ap_is_contiguous(idxs_ap.ap[1:])
    assert in_ap.ap[-1][1] == out_ap.ap[-1][1] == elem_size
    assert out_ap.ap[0][1] * out_ap.ap[1][1] == ((num_idxs + 127) // 128) * 128
    assert in_ap.ap[0][0] == elem_step
    stride_bytes = elem_step * mybir.dt.size(in_ap.dtype)
    stride_bytes_256 = _exact_div(stride_bytes, 256)
    assert stride_bytes_256 < 256
    _in_ap = eng.lower_ap_dma(in_ap, for_custom_bir_dma=True)
    _idxs_ap = eng.lower_ap(idxs_ap)
    _out_ap = eng.lower_ap(out_ap)
    return eng.add_instruction(
        mybir.InstDMAGatherAnt(
            name=eng.bass.get_next_instruction_name(),
            ins=[*_in_ap, _idxs_ap,
                 eng.lower_val_access(eng.to_reg(num_idxs))],
            outs=[_out_ap],
            transpose=False,
            num_idxs=num_idxs,
            elem_size=elem_size,
            stride_bytes_256=stride_bytes_256,
            gen_mode=0,
            single_packet=False,
            queue_num=queue_num,
            sbuf_tokens_per_rank=0,
            sbuf_free_dim_per_rank=0,
            sbuf_free_dim_pad_per_rank=0,
            sbuf_byte_offset=0,
        )
    )


# --------------------------------------------------------------------------
# device kernel builder
# --------------------------------------------------------------------------

def _build(npad, nslice, g_cnt, dg, total_slots):
    DT = mybir.dt
    fp32 = DT.float32
    bf16 = DT.bfloat16
    ACT = mybir.ActivationFunctionType
    base = 32768 if npad > 32768 else 0
    nc = bacc.Bacc("TRN2", target_bir_lowering=False, debug=False,
                   num_devices=NC, num_swdge_queues=NQ,
                   dynamic_dma_scratch_size=32768)

    xT = nc.dram_tensor("xT", [P, npad], bf16, kind="ExternalInput")
    xownT = nc.dram_tensor("xownT", [P, nslice], bf16, kind="ExternalInput")
    idx = nc.dram_tensor("idx", [P, total_slots // 16], DT.int16, kind="ExternalInput")
    wcat1 = nc.dram_tensor("wcat1", [P, 80], bf16, kind="ExternalInput")
    w2cat = nc.dram_tensor("w2cat", [2 * HF, 16], fp32, kind="ExternalInput")
    b1d = nc.dram_tensor("b1d", [HF], fp32, kind="ExternalInput")
    b2d = nc.dram_tensor("b2d", [C2], fp32, kind="ExternalInput")
    out = nc.dram_tensor("out", [nslice, C2], fp32, kind="ExternalOutput")

    tab1 = nc.dram_tensor("tab1", [npad + P, R1], bf16)
    tab2in = nc.dram_tensor("tab2in", [nslice, 8], fp32)
    tab2c = nc.dram_tensor("tab2c", [npad, 8], fp32, addr_space="Shared")
    tab2f = nc.dram_tensor("tab2f", [npad + P, R2], fp32)

    n_tiles = npad // P
    chunk = 6
    chunks = []
    t = 0
    while t < n_tiles:
        c = min(chunk, n_tiles - t)
        chunks.append((t, c))
        t += c
    offs = np.concatenate([[0], np.cumsum([P * d for d in dg])]).astype(int)

    with TileContext(nc) as tc:
        with (
            tc.tile_pool(name="persist", bufs=1) as pp,
            tc.tile_pool(name="pA_x", bufs=3) as pa_x,
            tc.tile_pool(name="pA_tab", bufs=4) as pa_tab,
            tc.tile_pool(name="pA_ps", bufs=2, space="PSUM") as pa_ps,
            tc.tile_pool(name="pB_g", bufs=7) as pb_g,
            tc.tile_pool(name="pIdx", bufs=8) as pidx,
            tc.tile_pool(name="pB_sc", bufs=4) as pb_sc,
            tc.tile_pool(name="pB_msg", bufs=2) as pb_msg,
            tc.tile_pool(name="pD_ps", bufs=2, space="PSUM") as pd_ps,
            tc.tile_pool(name="pD_ht", bufs=3) as pd_ht,
            tc.tile_pool(name="pE_g", bufs=8) as pe_g,
            tc.tile_pool(name="pE_sc", bufs=4) as pe_sc,
            tc.tile_pool(name="pM2", bufs=2) as pm2,
        ):
            # ---- persistent small tiles ----
            wcat1_sb = pp.tile([P, 80], bf16, tag="wcat1")
            nc.sync.dma_start(out=wcat1_sb[:], in_=wcat1[:])
            w2cat_sb = pp.tile([2 * HF, 16], fp32, tag="w2cat")
            nc.sync.dma_start(out=w2cat_sb[:], in_=w2cat[:])
            ones_sb = pp.tile([1, P], fp32, tag="ones")
            nc.vector.memset(ones_sb[:], 1.0)
            b1_sb = pp.tile([1, HF], fp32, tag="b1sb")
            nc.sync.dma_start(out=b1_sb[:], in_=b1d.ap().rearrange("(o c) -> o c", o=1))
            b2m_sb = pp.tile([1, 8], fp32, tag="b2msb")
            nc.vector.memset(b2m_sb[:], 0.0)
            nc.sync.dma_start(out=b2m_sb[:, 0:C2], in_=b2d.ap().rearrange("(o c) -> o c", o=1))
            ident_sb = pp.tile([P, P], fp32, tag="ident")
            make_identity(nc, ident_sb[:])

            b1bc_ps = pa_ps.tile([P, HF], fp32, tag="smallps")
            nc.tensor.matmul(out=b1bc_ps[:], lhsT=ones_sb[:], rhs=b1_sb[:], start=True, stop=True)
            b1bc_sb = pp.tile([P, HF], fp32, tag="b1bc")
            nc.vector.tensor_copy(out=b1bc_sb[:], in_=b1bc_ps[:])
            b2bc_ps = pa_ps.tile([P, 8], fp32, tag="smallps")
            nc.tensor.matmul(out=b2bc_ps[:], lhsT=ones_sb[:], rhs=b2m_sb[:], start=True, stop=True)
            b2bc_sb = pp.tile([P, 8], fp32, tag="b2bc")
            nc.vector.tensor_copy(out=b2bc_sb[:], in_=b2bc_ps[:])

            # ---- batched-tail persistent buffers ----
            strgown_sb = pp.tile([P, g_cnt * 8], fp32, tag="strgown")
            out1all = pp.tile([P, g_cnt * HF], fp32, tag="out1all")
            uuall = pp.tile([P, g_cnt * HF], fp32, tag="uuall")
            ssumall = pp.tile([P, g_cnt * 8], fp32, tag="ssumall")
            rinvall = pp.tile([P, g_cnt * 8], fp32, tag="rinvall")
            ps2all = pp.tile([P, g_cnt * 9], fp32, tag="ps2all")
            ssum2all = pp.tile([P, g_cnt], fp32, tag="ssum2all")
            rinv2all = pp.tile([P, g_cnt], fp32, tag="rinv2all")
            o2all = pp.tile([P, g_cnt * C2], fp32, tag="o2all")
            e7all = pp.tile([P, g_cnt * C2], fp32, tag="e7all")
            sum7all = pp.tile([P, g_cnt], fp32, tag="sum7all")
            r7all = pp.tile([P, g_cnt], fp32, tag="r7all")
            resall = pp.tile([P, g_cnt * C2], fp32, tag="resall")

            # ---- phase A: node table (replicated), bf16 rows of 256B:
            #      [proj64+b1 bf16 | s_src8 f32 | pad] ----
            for ci, (t0, cl) in enumerate(chunks):
                xc = pa_x.tile([P, chunk * P], bf16, tag="xchunk")
                nc.sync.dma_start(out=xc[:, 0:cl * P],
                                  in_=xT[:, t0 * P:(t0 + cl) * P])
                psC = pa_ps.tile([P, chunk * 80], fp32, tag="projps")
                for j in range(cl):
                    nc.tensor.matmul(out=psC[:, j * 80:(j + 1) * 80],
                                     lhsT=xc[:, j * P:(j + 1) * P],
                                     rhs=wcat1_sb[:], start=True, stop=True)
                psv = psC[:].rearrange("p (j c) -> p j c", c=80)
                tabt = pa_tab.tile([P, chunk * R1], bf16, tag="tabt")
                tbv = tabt[:].rearrange("p (j c) -> p j c", c=R1)
                nc.vector.tensor_copy(out=tbv[0:P, 0:cl, 0:HF], in_=psv[0:P, 0:cl, 0:HF])
                tbv32 = tabt[:].bitcast(fp32).rearrange("p (j c) -> p j c", c=R1 // 2)
                nc.vector.tensor_copy(out=tbv32[0:P, 0:cl, 32:40],
                                      in_=psv[0:P, 0:cl, HF:72])
                nc.sync.dma_start(
                    out=tab1.ap()[t0 * P:(t0 + cl) * P, 0:E1]
                    .rearrange("(j p) c -> p j c", p=P),
                    in_=tbv[0:P, 0:cl, 0:E1])

            # dedicated dummy row (= npad): zero payload, s_src = -150 so
            # pad edge slots contribute exp(-150) ~ 0 to every softmax
            dummy1 = pp.tile([1, R1], bf16, tag="dummy1")
            nc.vector.memset(dummy1[:], 0.0)
            nc.vector.memset(dummy1[:].bitcast(fp32)[:, 32:40], MASKVAL)
            nc.sync.dma_start(out=tab1.ap()[npad:npad + 1, :], in_=dummy1[:])

            # own-node s_trg1: bunches of 8 groups -> one PSUM tile + one copy
            g = 0
            while g < g_cnt:
                bl = min(8, g_cnt - g)
                pso = pa_ps.tile([P, 64], fp32, tag="smallps")
                for k in range(bl):
                    xog = pa_x.tile([P, P], bf16, tag="xog")
                    nc.sync.dma_start(out=xog[:], in_=xownT[:, (g + k) * P:(g + k + 1) * P])
                    nc.tensor.matmul(out=pso[:, k * 8:(k + 1) * 8], lhsT=xog[:],
                                     rhs=wcat1_sb[:, 72:80], start=True, stop=True)
                nc.vector.tensor_copy(out=strgown_sb[:, g * 8:(g + bl) * 8],
                                      in_=pso[:, 0:bl * 8])
                g += bl

            # ---- phase B: layer-1 groups (big ops only) ----
            for g in range(g_cnt):
                D = dg[g]
                L = P * D
                idxg = pidx.tile([P, (offs[g + 1] - offs[g]) // 16], DT.int16, tag="idxg")
                nc.sync.dma_start(out=idxg[:], in_=idx[:, offs[g] // 16:offs[g + 1] // 16])
                g1 = pb_g.tile([P, D * E1], bf16, tag="g1")
                _gather_raw(
                    nc.gpsimd,
                    out_ap=g1[:].rearrange("p (d c) -> p d c", c=E1),
                    in_ap=tab1[base:, 0:E1],
                    idxs_ap=idxg[:],
                    num_idxs=L, elem_size=E1, elem_step=R1,
                    queue_num=g % NQ)
                g1v = g1[:].rearrange("p (d c) -> p d c", c=E1)
                g1s = g1[:].bitcast(fp32).rearrange("p (d c) -> p d c", c=E1 // 2)

                sc = pb_sc.tile([P, D * 8], fp32, tag="scores")
                scv = sc[:].rearrange("p (d h) -> p d h", h=H1)
                strg_g = strgown_sb[:, g * 8:(g + 1) * 8]
                nc.vector.tensor_add(
                    out=scv, in0=g1s[:, :, 32:40],
                    in1=strg_g.rearrange("p (d h) -> p d h", d=1).to_broadcast([P, D, H1]))
                nc.vector.scalar_tensor_tensor(
                    out=sc[:], in0=sc[:], scalar=0.2, in1=sc[:],
                    op0=mybir.AluOpType.mult, op1=mybir.AluOpType.max)
                nc.scalar.activation(out=sc[:], in_=sc[:], func=ACT.Exp)
                nc.vector.tensor_reduce(
                    out=ssumall[:, g * 8:(g + 1) * 8],
                    in_=sc[:].rearrange("p (d h) -> p h d", h=H1),
                    axis=mybir.AxisListType.X, op=mybir.AluOpType.add)
                scb = pb_sc.tile([P, D * 8], bf16, tag="scb")
                nc.vector.tensor_copy(out=scb[:], in_=sc[:])

                msg = pb_msg.tile([P, D * HF], bf16, tag="msg")
                nc.vector.tensor_mul(
                    out=msg[:].rearrange("p (d h f) -> p d h f", h=H1, f=F1),
                    in0=g1v[:, :, 0:HF].rearrange("p d (h f) -> p d h f", f=F1),
                    in1=scb[:].rearrange("p (d h f) -> p d h f", h=H1, f=1
                                         ).to_broadcast([P, D, H1, F1]))
                nc.vector.tensor_reduce(
                    out=out1all[:, g * HF:(g + 1) * HF],
                    in_=msg[:].rearrange("p (d h f) -> p h f d", h=H1, f=F1),
                    axis=mybir.AxisListType.X, op=mybir.AluOpType.add)

            # ---- batched layer-1 tail + phase D, two halves for overlap ----
            def tail1(lo, hi):
                ng = hi - lo
                nc.vector.reciprocal(out=rinvall[:, lo * 8:hi * 8],
                                     in_=ssumall[:, lo * 8:hi * 8])
                o1 = out1all[:, lo * HF:hi * HF].rearrange(
                    "p (g h f) -> p g h f", h=H1, f=F1)
                nc.vector.tensor_mul(
                    out=o1, in0=o1,
                    in1=rinvall[:, lo * 8:hi * 8].rearrange(
                        "p (g h f) -> p g h f", h=H1, f=1
                    ).to_broadcast([P, ng, H1, F1]))
                # + b1, then ELU
                o = out1all[:, lo * HF:hi * HF]
                nc.vector.tensor_add(
                    out=o.rearrange("p (g c) -> p g c", c=HF),
                    in0=o.rearrange("p (g c) -> p g c", c=HF),
                    in1=b1bc_sb[:].rearrange("p (g c) -> p g c", g=1
                                             ).to_broadcast([P, ng, HF]))
                u = uuall[:, lo * HF:hi * HF]
                nc.vector.tensor_scalar_min(out=u, in0=o, scalar1=0.0)
                nc.scalar.activation(out=u, in_=u, func=ACT.Exp)
                nc.vector.tensor_scalar_max(out=o, in0=o, scalar1=0.0)
                nc.vector.tensor_add(out=o, in0=o, in1=u)
                nc.scalar.activation(out=o, in_=o, func=ACT.Copy, bias=-1.0)
                # transpose each group, project to layer-2 table rows
                for g in range(lo, hi):
                    psT = pd_ps.tile([HF, P], fp32, tag="psT")
                    nc.tensor.transpose(out=psT[:],
                                        in_=out1all[:, g * HF:(g + 1) * HF],
                                        identity=ident_sb[:])
                    hT = pd_ht.tile([HF, P], fp32, tag="hT")
                    nc.vector.tensor_copy(out=hT[:], in_=psT[:])
                    ps2 = pd_ps.tile([P, 9], fp32, tag="ps2")
                    nc.tensor.matmul(out=ps2[:], lhsT=hT[:],
                                     rhs=w2cat_sb[0:HF, 0:9],
                                     start=True, stop=True)
                    nc.vector.tensor_copy(out=ps2all[:, g * 9:(g + 1) * 9],
                                          in_=ps2[:])

            tail1(0, g_cnt // 2)
            tail1(g_cnt // 2, g_cnt)

            p2v = ps2all[:].rearrange("p (g c) -> p g c", c=9)
            nc.vector.tensor_add(
                out=p2v[:, :, 0:8], in0=p2v[:, :, 0:8],
                in1=b2bc_sb[:].rearrange("p (g c) -> p g c", g=1
                                         ).to_broadcast([P, g_cnt, 8]))

            # ---- phase C: exchange + expand layer-2 table ----
            nc.sync.dma_start(
                out=tab2in.ap().rearrange("(g p) c -> p g c", p=P),
                in_=p2v[:, :, 0:8])
            nc.gpsimd.collective_compute(
                "AllGather",
                mybir.AluOpType.bypass,
                ins=[tab2in[:]],
                outs=[tab2c[:]],
                replica_groups=[list(range(NC))],
            )
            nc.sync.dma_start(out=tab2f.ap()[0:npad, 0:8], in_=tab2c[:])
            dummy2 = pp.tile([1, 8], fp32, tag="dummy2")
            nc.vector.memset(dummy2[:], 0.0)
            nc.vector.memset(dummy2[:, 7:8], MASKVAL)
            nc.sync.dma_start(out=tab2f.ap()[npad:npad + 1, 0:8], in_=dummy2[:])

            # ---- phase E: layer 2 per group (big ops only) ----
            for g in range(g_cnt):
                D = dg[g]
                L = P * D
                idxg = pidx.tile([P, (offs[g + 1] - offs[g]) // 16], DT.int16, tag="idxg")
                nc.sync.dma_start(out=idxg[:], in_=idx[:, offs[g] // 16:offs[g + 1] // 16])
                g2 = pe_g.tile([P, D * E2], fp32, tag="g2")
                _gather_raw(
                    nc.gpsimd,
                    out_ap=g2[:].rearrange("p (d c) -> p d c", c=E2),
                    in_ap=tab2f[base:, 0:E2],
                    idxs_ap=idxg[:],
                    num_idxs=L, elem_size=E2, elem_step=R2,
                    queue_num=g % NQ)
                g2v = g2[:].rearrange("p (d c) -> p d c", c=E2)

                sc2 = pe_sc.tile([P, D], fp32, tag="sc2")
                nc.vector.tensor_scalar_add(
                    out=sc2[:],
                    in0=g2v[:, :, 7:8].rearrange("p d c -> p (d c)"),
                    scalar1=ps2all[:, g * 9 + 8:g * 9 + 9])
                nc.vector.scalar_tensor_tensor(
                    out=sc2[:], in0=sc2[:], scalar=0.2, in1=sc2[:],
                    op0=mybir.AluOpType.mult, op1=mybir.AluOpType.max)
                nc.scalar.activation(out=sc2[:], in_=sc2[:], func=ACT.Exp,
                                     accum_out=ssum2all[:, g:g + 1])
                m2 = pm2.tile([P, D * 8], fp32, tag="m2")
                nc.vector.tensor_mul(
                    out=m2[:].rearrange("p (d c) -> p d c", c=8),
                    in0=g2v[:, :, 0:8],
                    in1=sc2[:].rearrange("p (d c) -> p d c", c=1).to_broadcast([P, D, 8]))
                nc.vector.tensor_reduce(
                    out=o2all[:, g * C2:(g + 1) * C2],
                    in_=m2[:].rearrange("p (d c) -> p c d", c=8)[:, 0:C2, :],
                    axis=mybir.AxisListType.X, op=mybir.AluOpType.add)

            # ---- batched layer-2 tail: normalize + softmax + one DMA out ----
            nc.vector.reciprocal(out=rinv2all[:], in_=ssum2all[:])
            o2v = o2all[:].rearrange("p (g c) -> p g c", c=C2)
            nc.vector.tensor_mul(
                out=o2v, in0=o2v,
                in1=rinv2all[:].rearrange("p (g c) -> p g c", c=1
                                          ).to_broadcast([P, g_cnt, C2]))
            # softmax over the 7 logits (range-safe without max subtraction)
            nc.scalar.activation(out=e7all[:], in_=o2all[:], func=ACT.Exp)
            nc.vector.tensor_reduce(
                out=sum7all[:], in_=e7all[:].rearrange("p (g c) -> p g c", c=C2),
                axis=mybir.AxisListType.X, op=mybir.AluOpType.add)
            nc.vector.reciprocal(out=r7all[:], in_=sum7all[:])
            nc.vector.tensor_mul(
                out=resall[:].rearrange("p (g c) -> p g c", c=C2),
                in0=e7all[:].rearrange("p (g c) -> p g c", c=C2),
                in1=r7all[:].rearrange("p (g c) -> p g c", c=1
                                       ).to_broadcast([P, g_cnt, C2]))
            nc.sync.dma_start(
                out=out.ap().rearrange("(g p) c -> p g c", p=P),
                in_=resall[:].rearrange("p (g c) -> p g c", c=C2))

    nc.compile()
    return nc


# --------------------------------------------------------------------------
# host side
# --------------------------------------------------------------------------

def _preprocess(x, edge_index):
    src = np.asarray(edge_index[0], np.int64)
    trg = np.asarray(edge_index[1], np.int64)
    n = x.shape[0]
    e = src.shape[0]

    deg = np.bincount(trg, minlength=n)
    order = np.argsort(-deg, kind="stable")          # rank -> node
    g_cnt = math.ceil(n / (P * NC))
    if g_cnt * P * NC == n:
        g_cnt += 1  # ensure pad rows exist (dummy index must be a pad row)
    npad = g_cnt * P * NC
    nslice = g_cnt * P

    ranks = np.empty(n, np.int64)
    ranks[order] = np.arange(n)
    core_of = ranks % NC
    grp_of = ranks // (P * NC)
    slot_of = (ranks // NC) % P
    perm = core_of * nslice + grp_of * P + slot_of   # node -> perm position

    dg = []
    for g in range(g_cnt):
        w = order[P * NC * g: P * NC * (g + 1)]
        if len(w) == 0:
            dg.append(1)
            continue
        degs = deg[w]  # already descending
        dmax = max(int(degs.max()), 1)
        if len(degs) <= 1016 or int(degs[1016:].max()) == dmax:
            dmax += 1
        dg.append(dmax)
    offs = np.concatenate([[0], np.cumsum([P * d for d in dg])]).astype(np.int64)
    total_slots = int(offs[-1])

    dummy = npad  # the dedicated poison row appended to both tables
    base = 32768 if npad > 32768 else 0

    tp = perm[trg]
    eorder = np.argsort(tp, kind="stable")
    tps = tp[eorder]
    counts = np.bincount(tps, minlength=npad)
    starts = np.concatenate([[0], np.cumsum(counts)[:-1]])
    d_of = np.arange(e) - starts[tps]

    c_of = tps // nslice
    r_local = tps % nslice
    g_of = r_local // P
    p_of = r_local % P
    pos = offs[g_of] + d_of * P + p_of               # k = d*128 + p within group

    idx_flat = np.full((NC, total_slots), dummy - base, np.int16)
    idx_flat[c_of, pos] = (perm[src[eorder]] - base).astype(np.int16)

    idx_wrapped = np.empty((NC, P, total_slots // 16), np.int16)
    for g in range(g_cnt):
        lo, hi = int(offs[g]), int(offs[g + 1])
        blk = idx_flat[:, lo:hi].reshape(NC, (hi - lo) // 16, 16)  # [c, col, p16]
        wr = np.swapaxes(blk, 1, 2)                                # [c, p16, col]
        idx_wrapped[:, :, lo // 16:hi // 16] = np.tile(wr, (1, 8, 1))

    pad_mask = np.ones(npad, bool)
    pad_mask[perm] = False

    return dict(n=n, e=e, npad=npad, nslice=nslice, g_cnt=g_cnt, dg=dg,
                total_slots=total_slots, perm=perm, idx_wrapped=idx_wrapped,
                pad_mask=pad_mask, base=base)


def _prepare(x, edge_index, W1, a_src1, a_trg1, b1, W2, a_src2, a_trg2, b2):
    x = np.asarray(x, np.float32)
    W1 = np.asarray(W1, np.float32)
    a_src1 = np.asarray(a_src1, np.float32)
    a_trg1 = np.asarray(a_trg1, np.float32)
    b1 = np.asarray(b1, np.float32)
    W2 = np.asarray(W2, np.float32)
    a_src2 = np.asarray(a_src2, np.float32)
    a_trg2 = np.asarray(a_trg2, np.float32)
    b2 = np.asarray(b2, np.float32)

    meta = _preprocess(x, edge_index)
    npad, nslice, g_cnt = meta["npad"], meta["nslice"], meta["g_cnt"]
    perm = meta["perm"]

    xp = np.zeros((npad, FIN), np.float32)
    xp[perm] = x
    xT = np.ascontiguousarray(xp.T).astype(BF16)

    Wt = W1.T  # [128, 64], col = h*F + j
    w3 = W1.reshape(H1, F1, FIN)
    Asrc = np.einsum("hjf,hj->fh", w3, a_src1[0])
    Atrg = np.einsum("hjf,hj->fh", w3, a_trg1[0])
    wcat1 = np.concatenate([Wt, Asrc, Atrg], axis=1).astype(BF16)

    w2cat = np.zeros((HF, 16), np.float32)
    w2cat[:, 0:C2] = W2.T
    w2cat[:, C2] = W2.T @ a_src2[0, 0]
    w2cat[:, C2 + 1] = W2.T @ a_trg2[0, 0]
    w2cat = np.vstack([w2cat, w2cat])

    key = (npad, g_cnt, tuple(meta["dg"]))
    if key not in _CACHE:
        _CACHE[key] = _build(npad, nslice, g_cnt, meta["dg"], meta["total_slots"])
    nc = _CACHE[key]

    in_maps = []
    for c in range(NC):
        xownT = np.ascontiguousarray(xp[c * nslice:(c + 1) * nslice].T).astype(BF16)
        in_maps.append({
            "xT": xT,
            "xownT": xownT,
            "idx": np.ascontiguousarray(meta["idx_wrapped"][c]),
            "wcat1": wcat1,
            "w2cat": w2cat,
            "b1d": b1,
            "b2d": b2,
        })
    return nc, in_maps, meta


def kernel(x, edge_index, W1, a_src1, a_trg1, b1, W2, a_src2, a_trg2, b2):
    nc, in_maps, meta = _prepare(x, edge_index, W1, a_src1, a_trg1, b1,
                                 W2, a_src2, a_trg2, b2)
    res = run_bass_kernel_spmd(nc, in_maps, core_ids=list(range(NC)))
    full = np.concatenate([res.results[c]["out"] for c in range(NC)], axis=0)
    return full[meta["perm"]].astype(np.float32)


# revision 13
# speedup vs baseline: 2.6490x; 1.3242x over previous
"""2-layer GAT forward on 8 Trainium2 NeuronCores.

Strategy: target-node sharding (degree-sorted round-robin groups of 128).
v3: gathers on 4 SWDGE queues (4 desc-gen DSP pairs in parallel, enlarged
descriptor rings so gen overlaps drain), compact table rows (160B L1 /
32B L2) via a raw gather instruction, and a vector-engine diet: leaky-relu
and exp on the scalar engine, bias/mask folded into PE matmuls, per-group
tails (ELU, reciprocals, layer-2 projection, softmax) batched across groups.
"""

import math
import numpy as np
import ml_dtypes

import concourse.bass as bass
import concourse.mybir as mybir
from concourse import bacc
from concourse.tile import TileContext
from concourse.bass_utils import run_bass_kernel_spmd
from concourse.masks import make_identity
from concourse import ap_utils

BF16 = ml_dtypes.bfloat16

NC = 8
P = 128
FIN = 128
HF = 64
H1 = 8
F1 = 8
C2 = 7
R1 = 128    # tab1 row stride in bf16 elems (256B)
E1 = 80     # tab1 gathered elems (bf16): 64 proj + 16 (=8 f32 s_src)
R2 = 64     # tab2 row stride in f32 elems (256B)
E2 = 8      # tab2 gathered elems (f32)
MASKVAL = -150.0
NQ = 4      # SWDGE queues

_CACHE = {}


def _exact_div(a, b):
    assert a % b == 0
    return a // b


def _gather_raw(eng, out_ap, in_ap, idxs_ap, num_idxs, elem_size, elem_step,
                queue_num):
    """nc.gpsimd.dma_gather minus the %256 elem assert (transpose-only
    ucode restriction; non-transpose descriptors take arbitrary sizes)."""
    eng._assert_queue_num(queue_num)
    assert idxs_ap.dtype == mybir.dt.int16
    assert in_ap.dtype == out_ap.dtype
    assert in_ap.space == bass.MemorySpace.DRAM
    assert idxs_ap.space == bass.MemorySpace.SBUF
    assert out_ap.space == bass.MemorySpace.SBUF
    assert ap_utils.ap_is_contiguous(out_ap.ap[1:])
    assert ap_utils.ap_is_contiguous(idxs_ap.ap[1:])
    assert in_ap.ap[-1][1] == out_ap.ap[-1][1] == elem_size
    assert out_ap.ap[0][1] * out_ap.ap[1][1] == ((num_idxs + 127) // 128) * 128
    assert in_ap.ap[0][0] == elem_step
    stride_bytes = elem_step * mybir.dt.size(in_ap.dtype)
    stride_bytes_256 = _exact_div(stride_bytes, 256)
    assert stride_bytes_256 < 256
    _in_ap = eng.lower_ap_dma(in_ap, for_custom_bir_dma=True)
    _idxs_ap = eng.lower_ap(idxs_ap)
    _out_ap = eng.lower_ap(out_ap)
    return eng.add_instruction(
        mybir.InstDMAGatherAnt(
            name=eng.bass.get_next_instruction_name(),
            ins=[*_in_ap, _idxs_ap,
                 eng.lower_val_access(eng.to_reg(num_idxs))],
            outs=[_out_ap],
            transpose=False,
            num_idxs=num_idxs,
            elem_size=elem_size,
            stride_bytes_256=stride_bytes_256,
            gen_mode=0,
            single_packet=False,
            queue_num=queue_num,
            sbuf_tokens_per_rank=0,
            sbuf_free_dim_per_rank=0,
            sbuf_free_dim_pad_per_rank=0,
            sbuf_byte_offset=0,
        )
    )


# --------------------------------------------------------------------------
# device kernel builder
# --------------------------------------------------------------------------

def _build(npad, nslice, g_cnt, dg, total_slots):
    DT = mybir.dt
    fp32 = DT.float32
    bf16 = DT.bfloat16
    ACT = mybir.ActivationFunctionType
    base = 32768 if npad > 32768 else 0
    nc = bacc.Bacc("TRN2", target_bir_lowering=False, debug=False,
                   num_devices=NC, num_swdge_queues=NQ,
                   dynamic_dma_scratch_size=32768)

    xT = nc.dram_tensor("xT", [P, npad], bf16, kind="ExternalInput")
    xownT = nc.dram_tensor("xownT", [P, nslice], bf16, kind="ExternalInput")
    idx = nc.dram_tensor("idx", [P, total_slots // 16], DT.int16, kind="ExternalInput")
    wcat1 = nc.dram_tensor("wcat1", [P, 80], bf16, kind="ExternalInput")
    w2cat = nc.dram_tensor("w2cat", [2 * HF, 16], fp32, kind="ExternalInput")
    b1d = nc.dram_tensor("b1d", [HF], fp32, kind="ExternalInput")
    b2d = nc.dram_tensor("b2d", [C2], fp32, kind="ExternalInput")
    out = nc.dram_tensor("out", [nslice, C2], fp32, kind="ExternalOutput")

    tab1 = nc.dram_tensor("tab1", [npad + P, R1], bf16)
    tab2in = nc.dram_tensor("tab2in", [nslice, 8], fp32)
    tab2c = nc.dram_tensor("tab2c", [npad, 8], fp32, addr_space="Shared")
    tab2f = nc.dram_tensor("tab2f", [npad + P, R2], fp32)

    n_tiles = npad // P
    chunk = 6
    chunks = []
    t = 0
    while t < n_tiles:
        c = min(chunk, n_tiles - t)
        chunks.append((t, c))
        t += c
    offs = np.concatenate([[0], np.cumsum([P * d for d in dg])]).astype(int)

    with TileContext(nc) as tc:
        with (
            tc.tile_pool(name="persist", bufs=1) as pp,
            tc.tile_pool(name="pA_x", bufs=3) as pa_x,
            tc.tile_pool(name="pA_tab", bufs=4) as pa_tab,
            tc.tile_pool(name="pA_ps", bufs=2, space="PSUM") as pa_ps,
            tc.tile_pool(name="pB_g", bufs=7) as pb_g,
            tc.tile_pool(name="pIdx", bufs=8) as pidx,
            tc.tile_pool(name="pB_sc", bufs=4) as pb_sc,
            tc.tile_pool(name="pB_msg", bufs=2) as pb_msg,
            tc.tile_pool(name="pD_ps", bufs=2, space="PSUM") as pd_ps,
            tc.tile_pool(name="pD_ht", bufs=3) as pd_ht,
            tc.tile_pool(name="pE_g", bufs=8) as pe_g,
            tc.tile_pool(name="pE_sc", bufs=4) as pe_sc,
            tc.tile_pool(name="pM2", bufs=2) as pm2,
        ):
            # ---- persistent small tiles ----
            wcat1_sb = pp.tile([P, 80], bf16, tag="wcat1")
            nc.sync.dma_start(out=wcat1_sb[:], in_=wcat1[:])
            w2cat_sb = pp.tile([2 * HF, 16], fp32, tag="w2cat")
            nc.sync.dma_start(out=w2cat_sb[:], in_=w2cat[:])
            ones_sb = pp.tile([1, P], fp32, tag="ones")
            nc.vector.memset(ones_sb[:], 1.0)
            b1_sb = pp.tile([1, HF], fp32, tag="b1sb")
            nc.sync.dma_start(out=b1_sb[:], in_=b1d.ap().rearrange("(o c) -> o c", o=1))
            b2m_sb = pp.tile([1, 8], fp32, tag="b2msb")
            nc.vector.memset(b2m_sb[:], 0.0)
            nc.sync.dma_start(out=b2m_sb[:, 0:C2], in_=b2d.ap().rearrange("(o c) -> o c", o=1))
            ident_sb = pp.tile([P, P], fp32, tag="ident")
            make_identity(nc, ident_sb[:])

            b1bc_ps = pa_ps.tile([P, HF], fp32, tag="smallps")
            nc.tensor.matmul(out=b1bc_ps[:], lhsT=ones_sb[:], rhs=b1_sb[:], start=True, stop=True)
            b1bc_sb = pp.tile([P, HF], fp32, tag="b1bc")
            nc.vector.tensor_copy(out=b1bc_sb[:], in_=b1bc_ps[:])
            b2bc_ps = pa_ps.tile([P, 8], fp32, tag="smallps")
            nc.tensor.matmul(out=b2bc_ps[:], lhsT=ones_sb[:], rhs=b2m_sb[:], start=True, stop=True)
            b2bc_sb = pp.tile([P, 8], fp32, tag="b2bc")
            nc.vector.tensor_copy(out=b2bc_sb[:], in_=b2bc_ps[:])

            # ---- batched-tail persistent buffers ----
            strgown_sb = pp.tile([P, g_cnt * 8], fp32, tag="strgown")
            out1all = pp.tile([P, g_cnt * HF], fp32, tag="out1all")
            uuall = pp.tile([P, g_cnt * HF], fp32, tag="uuall")
            ssumall = pp.tile([P, g_cnt * 8], fp32, tag="ssumall")
            rinvall = pp.tile([P, g_cnt * 8], fp32, tag="rinvall")
            ps2all = pp.tile([P, g_cnt * 9], fp32, tag="ps2all")
            ssum2all = pp.tile([P, g_cnt], fp32, tag="ssum2all")
            rinv2all = pp.tile([P, g_cnt], fp32, tag="rinv2all")
            o2all = pp.tile([P, g_cnt * C2], fp32, tag="o2all")
            e7all = pp.tile([P, g_cnt * C2], fp32, tag="e7all")
            sum7all = pp.tile([P, g_cnt], fp32, tag="sum7all")
            r7all = pp.tile([P, g_cnt], fp32, tag="r7all")
            resall = pp.tile([P, g_cnt * C2], fp32, tag="resall")

            # ---- phase A: node table (replicated), bf16 rows of 256B:
            #      [proj64+b1 bf16 | s_src8 f32 | pad] ----
            for ci, (t0, cl) in enumerate(chunks):
                xc = pa_x.tile([P, chunk * P], bf16, tag="xchunk")
                nc.sync.dma_start(out=xc[:, 0:cl * P],
                                  in_=xT[:, t0 * P:(t0 + cl) * P])
                psC = pa_ps.tile([P, chunk * 80], fp32, tag="projps")
                for j in range(cl):
                    nc.tensor.matmul(out=psC[:, j * 80:(j + 1) * 80],
                                     lhsT=xc[:, j * P:(j + 1) * P],
                                     rhs=wcat1_sb[:], start=True, stop=True)
                psv = psC[:].rearrange("p (j c) -> p j c", c=80)
                tabt = pa_tab.tile([P, chunk * R1], bf16, tag="tabt")
                tbv = tabt[:].rearrange("p (j c) -> p j c", c=R1)
                nc.scalar.copy(out=tbv[0:P, 0:cl, 0:HF], in_=psv[0:P, 0:cl, 0:HF])
                tbv32 = tabt[:].bitcast(fp32).rearrange("p (j c) -> p j c", c=R1 // 2)
                nc.vector.tensor_copy(out=tbv32[0:P, 0:cl, 32:40],
                                      in_=psv[0:P, 0:cl, HF:72])
                nc.sync.dma_start(
                    out=tab1.ap()[t0 * P:(t0 + cl) * P, 0:E1]
                    .rearrange("(j p) c -> p j c", p=P),
                    in_=tbv[0:P, 0:cl, 0:E1])

            # dedicated dummy row (= npad): zero payload, s_src = -150 so
            # pad edge slots contribute exp(-150) ~ 0 to every softmax
            dummy1 = pp.tile([1, R1], bf16, tag="dummy1")
            nc.vector.memset(dummy1[:], 0.0)
            nc.vector.memset(dummy1[:].bitcast(fp32)[:, 32:40], MASKVAL)
            nc.sync.dma_start(out=tab1.ap()[npad:npad + 1, :], in_=dummy1[:])

            # own-node s_trg1: bunches of 8 groups -> one PSUM tile + one copy
            g = 0
            while g < g_cnt:
                bl = min(8, g_cnt - g)
                pso = pa_ps.tile([P, 64], fp32, tag="smallps")
                for k in range(bl):
                    xog = pa_x.tile([P, P], bf16, tag="xog")
                    nc.sync.dma_start(out=xog[:], in_=xownT[:, (g + k) * P:(g + k + 1) * P])
                    nc.tensor.matmul(out=pso[:, k * 8:(k + 1) * 8], lhsT=xog[:],
                                     rhs=wcat1_sb[:, 72:80], start=True, stop=True)
                nc.vector.tensor_copy(out=strgown_sb[:, g * 8:(g + bl) * 8],
                                      in_=pso[:, 0:bl * 8])
                g += bl

            # ---- phase B: layer-1 groups (big ops only) ----
            for g in range(g_cnt):
                D = dg[g]
                L = P * D
                idxg = pidx.tile([P, (offs[g + 1] - offs[g]) // 16], DT.int16, tag="idxg")
                nc.sync.dma_start(out=idxg[:], in_=idx[:, offs[g] // 16:offs[g + 1] // 16])
                g1 = pb_g.tile([P, D * E1], bf16, tag="g1")
                _gather_raw(
                    nc.gpsimd,
                    out_ap=g1[:].rearrange("p (d c) -> p d c", c=E1),
                    in_ap=tab1[base:, 0:E1],
                    idxs_ap=idxg[:],
                    num_idxs=L, elem_size=E1, elem_step=R1,
                    queue_num=g % NQ)
                g1v = g1[:].rearrange("p (d c) -> p d c", c=E1)
                g1s = g1[:].bitcast(fp32).rearrange("p (d c) -> p d c", c=E1 // 2)

                sc = pb_sc.tile([P, D * 8], fp32, tag="scores")
                scv = sc[:].rearrange("p (d h) -> p d h", h=H1)
                strg_g = strgown_sb[:, g * 8:(g + 1) * 8]
                nc.vector.tensor_add(
                    out=scv, in0=g1s[:, :, 32:40],
                    in1=strg_g.rearrange("p (d h) -> p d h", d=1).to_broadcast([P, D, H1]))
                nc.vector.scalar_tensor_tensor(
                    out=sc[:], in0=sc[:], scalar=0.2, in1=sc[:],
                    op0=mybir.AluOpType.mult, op1=mybir.AluOpType.max)
                scb = pb_sc.tile([P, D * 8], bf16, tag="scb")
                nc.scalar.activation(out=scb[:], in_=sc[:], func=ACT.Exp)
                nc.vector.tensor_reduce(
                    out=ssumall[:, g * 8:(g + 1) * 8],
                    in_=scb[:].rearrange("p (d h) -> p h d", h=H1),
                    axis=mybir.AxisListType.X, op=mybir.AluOpType.add)

                msg = pb_msg.tile([P, D * HF], bf16, tag="msg")
                nc.vector.tensor_mul(
                    out=msg[:].rearrange("p (d h f) -> p d h f", h=H1, f=F1),
                    in0=g1v[:, :, 0:HF].rearrange("p d (h f) -> p d h f", f=F1),
                    in1=scb[:].rearrange("p (d h f) -> p d h f", h=H1, f=1
                                         ).to_broadcast([P, D, H1, F1]))
                nc.vector.tensor_reduce(
                    out=out1all[:, g * HF:(g + 1) * HF],
                    in_=msg[:].rearrange("p (d h f) -> p h f d", h=H1, f=F1),
                    axis=mybir.AxisListType.X, op=mybir.AluOpType.add)

            # ---- batched layer-1 tail + phase D, two halves for overlap ----
            def tail1(lo, hi):
                ng = hi - lo
                nc.vector.reciprocal(out=rinvall[:, lo * 8:hi * 8],
                                     in_=ssumall[:, lo * 8:hi * 8])
                o1 = out1all[:, lo * HF:hi * HF].rearrange(
                    "p (g h f) -> p g h f", h=H1, f=F1)
                nc.vector.tensor_mul(
                    out=o1, in0=o1,
                    in1=rinvall[:, lo * 8:hi * 8].rearrange(
                        "p (g h f) -> p g h f", h=H1, f=1
                    ).to_broadcast([P, ng, H1, F1]))
                # + b1, then ELU
                o = out1all[:, lo * HF:hi * HF]
                nc.vector.tensor_add(
                    out=o.rearrange("p (g c) -> p g c", c=HF),
                    in0=o.rearrange("p (g c) -> p g c", c=HF),
                    in1=b1bc_sb[:].rearrange("p (g c) -> p g c", g=1
                                             ).to_broadcast([P, ng, HF]))
                u = uuall[:, lo * HF:hi * HF]
                nc.vector.tensor_scalar_min(out=u, in0=o, scalar1=0.0)
                nc.scalar.activation(out=u, in_=u, func=ACT.Exp)
                nc.vector.tensor_scalar_max(out=o, in0=o, scalar1=0.0)
                nc.vector.tensor_add(out=o, in0=o, in1=u)
                nc.scalar.activation(out=o, in_=o, func=ACT.Copy, bias=-1.0)
                # transpose each group, project to layer-2 table rows
                for g in range(lo, hi):
                    psT = pd_ps.tile([HF, P], fp32, tag="psT")
                    nc.tensor.transpose(out=psT[:],
                                        in_=out1all[:, g * HF:(g + 1) * HF],
                                        identity=ident_sb[:])
                    hT = pd_ht.tile([HF, P], fp32, tag="hT")
                    nc.vector.tensor_copy(out=hT[:], in_=psT[:])
                    ps2 = pd_ps.tile([P, 9], fp32, tag="ps2")
                    nc.tensor.matmul(out=ps2[:], lhsT=hT[:],
                                     rhs=w2cat_sb[0:HF, 0:9],
                                     start=True, stop=True)
                    nc.vector.tensor_copy(out=ps2all[:, g * 9:(g + 1) * 9],
                                          in_=ps2[:])

            tail1(0, g_cnt // 2)
            tail1(g_cnt // 2, g_cnt)

            p2v = ps2all[:].rearrange("p (g c) -> p g c", c=9)
            nc.vector.tensor_add(
                out=p2v[:, :, 0:8], in0=p2v[:, :, 0:8],
                in1=b2bc_sb[:].rearrange("p (g c) -> p g c", g=1
                                         ).to_broadcast([P, g_cnt, 8]))

            # ---- phase C: exchange + expand layer-2 table ----
            nc.sync.dma_start(
                out=tab2in.ap().rearrange("(g p) c -> p g c", p=P),
                in_=p2v[:, :, 0:8])
            nc.gpsimd.collective_compute(
                "AllGather",
                mybir.AluOpType.bypass,
                ins=[tab2in[:]],
                outs=[tab2c[:]],
                replica_groups=[list(range(NC))],
            )
            nc.sync.dma_start(out=tab2f.ap()[0:npad, 0:8], in_=tab2c[:])
            dummy2 = pp.tile([1, 8], fp32, tag="dummy2")
            nc.vector.memset(dummy2[:], 0.0)
            nc.vector.memset(dummy2[:, 7:8], MASKVAL)
            nc.sync.dma_start(out=tab2f.ap()[npad:npad + 1, 0:8], in_=dummy2[:])

            # ---- phase E: layer 2 per group (big ops only) ----
            for g in range(g_cnt):
                D = dg[g]
                L = P * D
                idxg = pidx.tile([P, (offs[g + 1] - offs[g]) // 16], DT.int16, tag="idxg")
                nc.sync.dma_start(out=idxg[:], in_=idx[:, offs[g] // 16:offs[g + 1] // 16])
                g2 = pe_g.tile([P, D * E2], fp32, tag="g2")
                _gather_raw(
                    nc.gpsimd,
                    out_ap=g2[:].rearrange("p (d c) -> p d c", c=E2),
                    in_ap=tab2f[base:, 0:E2],
                    idxs_ap=idxg[:],
                    num_idxs=L, elem_size=E2, elem_step=R2,
                    queue_num=g % NQ)
                g2v = g2[:].rearrange("p (d c) -> p d c", c=E2)

                sc2 = pe_sc.tile([P, D], fp32, tag="sc2")
                nc.vector.tensor_scalar_add(
                    out=sc2[:],
                    in0=g2v[:, :, 7:8].rearrange("p d c -> p (d c)"),
                    scalar1=ps2all[:, g * 9 + 8:g * 9 + 9])
                nc.vector.scalar_tensor_tensor(
                    out=sc2[:], in0=sc2[:], scalar=0.2, in1=sc2[:],
                    op0=mybir.AluOpType.mult, op1=mybir.AluOpType.max)
                nc.scalar.activation(out=sc2[:], in_=sc2[:], func=ACT.Exp,
                                     accum_out=ssum2all[:, g:g + 1])
                m2 = pm2.tile([P, D * 8], fp32, tag="m2")
                nc.vector.tensor_mul(
                    out=m2[:].rearrange("p (d c) -> p d c", c=8),
                    in0=g2v[:, :, 0:8],
                    in1=sc2[:].rearrange("p (d c) -> p d c", c=1).to_broadcast([P, D, 8]))
                nc.vector.tensor_reduce(
                    out=o2all[:, g * C2:(g + 1) * C2],
                    in_=m2[:].rearrange("p (d c) -> p c d", c=8)[:, 0:C2, :],
                    axis=mybir.AxisListType.X, op=mybir.AluOpType.add)

            # ---- batched layer-2 tail: normalize + softmax + one DMA out ----
            nc.vector.reciprocal(out=rinv2all[:], in_=ssum2all[:])
            o2v = o2all[:].rearrange("p (g c) -> p g c", c=C2)
            nc.vector.tensor_mul(
                out=o2v, in0=o2v,
                in1=rinv2all[:].rearrange("p (g c) -> p g c", c=1
                                          ).to_broadcast([P, g_cnt, C2]))
            # softmax over the 7 logits (range-safe without max subtraction)
            nc.scalar.activation(out=e7all[:], in_=o2all[:], func=ACT.Exp)
            nc.vector.tensor_reduce(
                out=sum7all[:], in_=e7all[:].rearrange("p (g c) -> p g c", c=C2),
                axis=mybir.AxisListType.X, op=mybir.AluOpType.add)
            nc.vector.reciprocal(out=r7all[:], in_=sum7all[:])
            nc.vector.tensor_mul(
                out=resall[:].rearrange("p (g c) -> p g c", c=C2),
                in0=e7all[:].rearrange("p (g c) -> p g c", c=C2),
                in1=r7all[:].rearrange("p (g c) -> p g c", c=1
                                       ).to_broadcast([P, g_cnt, C2]))
            nc.sync.dma_start(
                out=out.ap().rearrange("(g p) c -> p g c", p=P),
                in_=resall[:].rearrange("p (g c) -> p g c", c=C2))

    nc.compile()
    return nc


# --------------------------------------------------------------------------
# host side
# --------------------------------------------------------------------------

def _preprocess(x, edge_index):
    src = np.asarray(edge_index[0], np.int64)
    trg = np.asarray(edge_index[1], np.int64)
    n = x.shape[0]
    e = src.shape[0]

    deg = np.bincount(trg, minlength=n)
    order = np.argsort(-deg, kind="stable")          # rank -> node
    g_cnt = math.ceil(n / (P * NC))
    if g_cnt * P * NC == n:
        g_cnt += 1  # ensure pad rows exist (dummy index must be a pad row)
    npad = g_cnt * P * NC
    nslice = g_cnt * P

    ranks = np.empty(n, np.int64)
    ranks[order] = np.arange(n)
    core_of = ranks % NC
    grp_of = ranks // (P * NC)
    slot_of = (ranks // NC) % P
    perm = core_of * nslice + grp_of * P + slot_of   # node -> perm position

    dg = []
    for g in range(g_cnt):
        w = order[P * NC * g: P * NC * (g + 1)]
        if len(w) == 0:
            dg.append(1)
            continue
        degs = deg[w]  # already descending
        dmax = max(int(degs.max()), 1)
        if len(degs) <= 1016 or int(degs[1016:].max()) == dmax:
            dmax += 1
        dg.append(dmax)
    offs = np.concatenate([[0], np.cumsum([P * d for d in dg])]).astype(np.int64)
    total_slots = int(offs[-1])

    dummy = npad  # the dedicated poison row appended to both tables
    base = 32768 if npad > 32768 else 0

    tp = perm[trg]
    eorder = np.argsort(tp, kind="stable")
    tps = tp[eorder]
    counts = np.bincount(tps, minlength=npad)
    starts = np.concatenate([[0], np.cumsum(counts)[:-1]])
    d_of = np.arange(e) - starts[tps]

    c_of = tps // nslice
    r_local = tps % nslice
    g_of = r_local // P
    p_of = r_local % P
    pos = offs[g_of] + d_of * P + p_of               # k = d*128 + p within group

    idx_flat = np.full((NC, total_slots), dummy - base, np.int16)
    idx_flat[c_of, pos] = (perm[src[eorder]] - base).astype(np.int16)

    idx_wrapped = np.empty((NC, P, total_slots // 16), np.int16)
    for g in range(g_cnt):
        lo, hi = int(offs[g]), int(offs[g + 1])
        blk = idx_flat[:, lo:hi].reshape(NC, (hi - lo) // 16, 16)  # [c, col, p16]
        wr = np.swapaxes(blk, 1, 2)                                # [c, p16, col]
        idx_wrapped[:, :, lo // 16:hi // 16] = np.tile(wr, (1, 8, 1))

    pad_mask = np.ones(npad, bool)
    pad_mask[perm] = False

    return dict(n=n, e=e, npad=npad, nslice=nslice, g_cnt=g_cnt, dg=dg,
                total_slots=total_slots, perm=perm, idx_wrapped=idx_wrapped,
                pad_mask=pad_mask, base=base)


def _prepare(x, edge_index, W1, a_src1, a_trg1, b1, W2, a_src2, a_trg2, b2):
    x = np.asarray(x, np.float32)
    W1 = np.asarray(W1, np.float32)
    a_src1 = np.asarray(a_src1, np.float32)
    a_trg1 = np.asarray(a_trg1, np.float32)
    b1 = np.asarray(b1, np.float32)
    W2 = np.asarray(W2, np.float32)
    a_src2 = np.asarray(a_src2, np.float32)
    a_trg2 = np.asarray(a_trg2, np.float32)
    b2 = np.asarray(b2, np.float32)

    meta = _preprocess(x, edge_index)
    npad, nslice, g_cnt = meta["npad"], meta["nslice"], meta["g_cnt"]
    perm = meta["perm"]

    xp = np.zeros((npad, FIN), np.float32)
    xp[perm] = x
    xT = np.ascontiguousarray(xp.T).astype(BF16)

    Wt = W1.T  # [128, 64], col = h*F + j
    w3 = W1.reshape(H1, F1, FIN)
    Asrc = np.einsum("hjf,hj->fh", w3, a_src1[0])
    Atrg = np.einsum("hjf,hj->fh", w3, a_trg1[0])
    wcat1 = np.concatenate([Wt, Asrc, Atrg], axis=1).astype(BF16)

    w2cat = np.zeros((HF, 16), np.float32)
    w2cat[:, 0:C2] = W2.T
    w2cat[:, C2] = W2.T @ a_src2[0, 0]
    w2cat[:, C2 + 1] = W2.T @ a_trg2[0, 0]
    w2cat = np.vstack([w2cat, w2cat])

    key = (npad, g_cnt, tuple(meta["dg"]))
    if key not in _CACHE:
        _CACHE[key] = _build(npad, nslice, g_cnt, meta["dg"], meta["total_slots"])
    nc = _CACHE[key]

    in_maps = []
    for c in range(NC):
        xownT = np.ascontiguousarray(xp[c * nslice:(c + 1) * nslice].T).astype(BF16)
        in_maps.append({
            "xT": xT,
            "xownT": xownT,
            "idx": np.ascontiguousarray(meta["idx_wrapped"][c]),
            "wcat1": wcat1,
            "w2cat": w2cat,
            "b1d": b1,
            "b2d": b2,
        })
    return nc, in_maps, meta


def kernel(x, edge_index, W1, a_src1, a_trg1, b1, W2, a_src2, a_trg2, b2):
    nc, in_maps, meta = _prepare(x, edge_index, W1, a_src1, a_trg1, b1,
                                 W2, a_src2, a_trg2, b2)
    res = run_bass_kernel_spmd(nc, in_maps, core_ids=list(range(NC)))
    full = np.concatenate([res.results[c]["out"] for c in range(NC)], axis=0)
    return full[meta["perm"]].astype(np.float32)


# revision 15
# speedup vs baseline: 2.6525x; 1.0013x over previous
"""2-layer GAT forward on 8 Trainium2 NeuronCores.

Strategy: target-node sharding (degree-sorted round-robin groups of 128).
v3: gathers on 4 SWDGE queues (4 desc-gen DSP pairs in parallel, enlarged
descriptor rings so gen overlaps drain), compact table rows (160B L1 /
32B L2) via a raw gather instruction, and a vector-engine diet: leaky-relu
and exp on the scalar engine, bias/mask folded into PE matmuls, per-group
tails (ELU, reciprocals, layer-2 projection, softmax) batched across groups.
"""

import math
import numpy as np
import ml_dtypes

import concourse.bass as bass
import concourse.mybir as mybir
from concourse import bacc
from concourse.tile import TileContext
from concourse.bass_utils import run_bass_kernel_spmd
from concourse.masks import make_identity
from concourse import ap_utils

BF16 = ml_dtypes.bfloat16

NC = 8
P = 128
FIN = 128
HF = 64
H1 = 8
F1 = 8
C2 = 7
R1 = 128    # tab1 row stride in bf16 elems (256B)
E1 = 80     # tab1 gathered elems (bf16): 64 proj + 16 (=8 f32 s_src)
R2 = 64     # tab2 row stride in f32 elems (256B)
E2 = 8      # tab2 gathered elems (f32)
MASKVAL = -150.0
NQ = 4      # SWDGE queues

_CACHE = {}


def _exact_div(a, b):
    assert a % b == 0
    return a // b


def _gather_raw(eng, out_ap, in_ap, idxs_ap, num_idxs, elem_size, elem_step,
                queue_num):
    """nc.gpsimd.dma_gather minus the %256 elem assert (transpose-only
    ucode restriction; non-transpose descriptors take arbitrary sizes)."""
    eng._assert_queue_num(queue_num)
    assert idxs_ap.dtype == mybir.dt.int16
    assert in_ap.dtype == out_ap.dtype
    assert in_ap.space == bass.MemorySpace.DRAM
    assert idxs_ap.space == bass.MemorySpace.SBUF
    assert out_ap.space == bass.MemorySpace.SBUF
    assert ap_utils.ap_is_contiguous(out_ap.ap[1:])
    assert ap_utils.ap_is_contiguous(idxs_ap.ap[1:])
    assert in_ap.ap[-1][1] == out_ap.ap[-1][1] == elem_size
    assert out_ap.ap[0][1] * out_ap.ap[1][1] == ((num_idxs + 127) // 128) * 128
    assert in_ap.ap[0][0] == elem_step
    stride_bytes = elem_step * mybir.dt.size(in_ap.dtype)
    stride_bytes_256 = _exact_div(stride_bytes, 256)
    assert stride_bytes_256 < 256
    _in_ap = eng.lower_ap_dma(in_ap, for_custom_bir_dma=True)
    _idxs_ap = eng.lower_ap(idxs_ap)
    _out_ap = eng.lower_ap(out_ap)
    return eng.add_instruction(
        mybir.InstDMAGatherAnt(
            name=eng.bass.get_next_instruction_name(),
            ins=[*_in_ap, _idxs_ap,
                 eng.lower_val_access(eng.to_reg(num_idxs))],
            outs=[_out_ap],
            transpose=False,
            num_idxs=num_idxs,
            elem_size=elem_size,
            stride_bytes_256=stride_bytes_256,
            gen_mode=0,
            single_packet=False,
            queue_num=queue_num,
            sbuf_tokens_per_rank=0,
            sbuf_free_dim_per_rank=0,
            sbuf_free_dim_pad_per_rank=0,
            sbuf_byte_offset=0,
        )
    )


# --------------------------------------------------------------------------
# device kernel builder
# --------------------------------------------------------------------------

def _build(npad, nslice, g_cnt, dg, total_slots):
    DT = mybir.dt
    fp32 = DT.float32
    bf16 = DT.bfloat16
    ACT = mybir.ActivationFunctionType
    base = 32768 if npad > 32768 else 0
    nc = bacc.Bacc("TRN2", target_bir_lowering=False, debug=False,
                   num_devices=NC, num_swdge_queues=NQ,
                   dynamic_dma_scratch_size=32768)

    xT = nc.dram_tensor("xT", [P, npad], bf16, kind="ExternalInput")
    xownT = nc.dram_tensor("xownT", [P, nslice], bf16, kind="ExternalInput")
    idx = nc.dram_tensor("idx", [P, total_slots // 16], DT.int16, kind="ExternalInput")
    wcat1 = nc.dram_tensor("wcat1", [P, 80], bf16, kind="ExternalInput")
    w2cat = nc.dram_tensor("w2cat", [2 * HF, 16], fp32, kind="ExternalInput")
    b1d = nc.dram_tensor("b1d", [HF], fp32, kind="ExternalInput")
    b2d = nc.dram_tensor("b2d", [C2], fp32, kind="ExternalInput")
    out = nc.dram_tensor("out", [nslice, C2], fp32, kind="ExternalOutput")

    tab1 = nc.dram_tensor("tab1", [npad + P, R1], bf16)
    tab2in = nc.dram_tensor("tab2in", [nslice, 8], fp32)
    gmid = (g_cnt * 3) // 4
    nsA = gmid * P
    nsB = nslice - nsA
    tab2cA = nc.dram_tensor("tab2cA", [NC * nsA, 8], fp32, addr_space="Shared")
    tab2cB = nc.dram_tensor("tab2cB", [NC * nsB, 8], fp32, addr_space="Shared")
    tab2f = nc.dram_tensor("tab2f", [npad + P, R2], fp32)

    n_tiles = npad // P
    chunk = 6
    chunks = []
    t = 0
    while t < n_tiles:
        c = min(chunk, n_tiles - t)
        chunks.append((t, c))
        t += c
    offs = np.concatenate([[0], np.cumsum([P * d for d in dg])]).astype(int)

    with TileContext(nc) as tc:
        with (
            tc.tile_pool(name="persist", bufs=1) as pp,
            tc.tile_pool(name="pA_x", bufs=3) as pa_x,
            tc.tile_pool(name="pA_tab", bufs=4) as pa_tab,
            tc.tile_pool(name="pA_ps", bufs=2, space="PSUM") as pa_ps,
            tc.tile_pool(name="pB_g", bufs=7) as pb_g,
            tc.tile_pool(name="pIdx", bufs=8) as pidx,
            tc.tile_pool(name="pB_sc", bufs=4) as pb_sc,
            tc.tile_pool(name="pB_msg", bufs=2) as pb_msg,
            tc.tile_pool(name="pD_ps", bufs=2, space="PSUM") as pd_ps,
            tc.tile_pool(name="pD_ht", bufs=3) as pd_ht,
            tc.tile_pool(name="pE_g", bufs=8) as pe_g,
            tc.tile_pool(name="pE_sc", bufs=4) as pe_sc,
            tc.tile_pool(name="pM2", bufs=2) as pm2,
        ):
            # ---- persistent small tiles ----
            wcat1_sb = pp.tile([P, 80], bf16, tag="wcat1")
            nc.sync.dma_start(out=wcat1_sb[:], in_=wcat1[:])
            w2cat_sb = pp.tile([2 * HF, 16], fp32, tag="w2cat")
            nc.sync.dma_start(out=w2cat_sb[:], in_=w2cat[:])
            ones_sb = pp.tile([1, P], fp32, tag="ones")
            nc.vector.memset(ones_sb[:], 1.0)
            b1_sb = pp.tile([1, HF], fp32, tag="b1sb")
            nc.sync.dma_start(out=b1_sb[:], in_=b1d.ap().rearrange("(o c) -> o c", o=1))
            b2m_sb = pp.tile([1, 8], fp32, tag="b2msb")
            nc.vector.memset(b2m_sb[:], 0.0)
            nc.sync.dma_start(out=b2m_sb[:, 0:C2], in_=b2d.ap().rearrange("(o c) -> o c", o=1))
            ident_sb = pp.tile([P, P], fp32, tag="ident")
            make_identity(nc, ident_sb[:])

            b1bc_ps = pa_ps.tile([P, HF], fp32, tag="smallps")
            nc.tensor.matmul(out=b1bc_ps[:], lhsT=ones_sb[:], rhs=b1_sb[:], start=True, stop=True)
            b1bc_sb = pp.tile([P, HF], fp32, tag="b1bc")
            nc.vector.tensor_copy(out=b1bc_sb[:], in_=b1bc_ps[:])
            b2bc_ps = pa_ps.tile([P, 8], fp32, tag="smallps")
            nc.tensor.matmul(out=b2bc_ps[:], lhsT=ones_sb[:], rhs=b2m_sb[:], start=True, stop=True)
            b2bc_sb = pp.tile([P, 8], fp32, tag="b2bc")
            nc.vector.tensor_copy(out=b2bc_sb[:], in_=b2bc_ps[:])

            # ---- batched-tail persistent buffers ----
            strgown_sb = pp.tile([P, g_cnt * 8], fp32, tag="strgown")
            out1all = pp.tile([P, g_cnt * HF], fp32, tag="out1all")
            uuall = pp.tile([P, g_cnt * HF], fp32, tag="uuall")
            ssumall = pp.tile([P, g_cnt * 8], fp32, tag="ssumall")
            rinvall = pp.tile([P, g_cnt * 8], fp32, tag="rinvall")
            ps2all = pp.tile([P, g_cnt * 9], fp32, tag="ps2all")
            ssum2all = pp.tile([P, g_cnt], fp32, tag="ssum2all")
            rinv2all = pp.tile([P, g_cnt], fp32, tag="rinv2all")
            o2all = pp.tile([P, g_cnt * C2], fp32, tag="o2all")
            e7all = pp.tile([P, g_cnt * C2], fp32, tag="e7all")
            sum7all = pp.tile([P, g_cnt], fp32, tag="sum7all")
            r7all = pp.tile([P, g_cnt], fp32, tag="r7all")
            resall = pp.tile([P, g_cnt * C2], fp32, tag="resall")

            # ---- phase A: node table (replicated), bf16 rows of 256B:
            #      [proj64+b1 bf16 | s_src8 f32 | pad] ----
            for ci, (t0, cl) in enumerate(chunks):
                xc = pa_x.tile([P, chunk * P], bf16, tag="xchunk")
                nc.sync.dma_start(out=xc[:, 0:cl * P],
                                  in_=xT[:, t0 * P:(t0 + cl) * P])
                psC = pa_ps.tile([P, chunk * 80], fp32, tag="projps")
                for j in range(cl):
                    nc.tensor.matmul(out=psC[:, j * 80:(j + 1) * 80],
                                     lhsT=xc[:, j * P:(j + 1) * P],
                                     rhs=wcat1_sb[:], start=True, stop=True)
                psv = psC[:].rearrange("p (j c) -> p j c", c=80)
                tabt = pa_tab.tile([P, chunk * R1], bf16, tag="tabt")
                tbv = tabt[:].rearrange("p (j c) -> p j c", c=R1)
                nc.scalar.copy(out=tbv[0:P, 0:cl, 0:HF], in_=psv[0:P, 0:cl, 0:HF])
                tbv32 = tabt[:].bitcast(fp32).rearrange("p (j c) -> p j c", c=R1 // 2)
                nc.vector.tensor_copy(out=tbv32[0:P, 0:cl, 32:40],
                                      in_=psv[0:P, 0:cl, HF:72])
                nc.sync.dma_start(
                    out=tab1.ap()[t0 * P:(t0 + cl) * P, 0:E1]
                    .rearrange("(j p) c -> p j c", p=P),
                    in_=tbv[0:P, 0:cl, 0:E1])

            # dedicated dummy row (= npad): zero payload, s_src = -150 so
            # pad edge slots contribute exp(-150) ~ 0 to every softmax
            dummy1 = pp.tile([1, R1], bf16, tag="dummy1")
            nc.vector.memset(dummy1[:], 0.0)
            nc.vector.memset(dummy1[:].bitcast(fp32)[:, 32:40], MASKVAL)
            nc.sync.dma_start(out=tab1.ap()[npad:npad + 1, :], in_=dummy1[:])

            # own-node s_trg1: bunches of 8 groups -> one PSUM tile + one copy
            g = 0
            while g < g_cnt:
                bl = min(8, g_cnt - g)
                pso = pa_ps.tile([P, 64], fp32, tag="smallps")
                for k in range(bl):
                    xog = pa_x.tile([P, P], bf16, tag="xog")
                    nc.sync.dma_start(out=xog[:], in_=xownT[:, (g + k) * P:(g + k + 1) * P])
                    nc.tensor.matmul(out=pso[:, k * 8:(k + 1) * 8], lhsT=xog[:],
                                     rhs=wcat1_sb[:, 72:80], start=True, stop=True)
                nc.vector.tensor_copy(out=strgown_sb[:, g * 8:(g + bl) * 8],
                                      in_=pso[:, 0:bl * 8])
                g += bl

            # ---- phase B: layer-1 groups (big ops only) ----
            for g in range(g_cnt):
                D = dg[g]
                L = P * D
                idxg = pidx.tile([P, (offs[g + 1] - offs[g]) // 16], DT.int16, tag="idxg")
                nc.sync.dma_start(out=idxg[:], in_=idx[:, offs[g] // 16:offs[g + 1] // 16])
                g1 = pb_g.tile([P, D * E1], bf16, tag="g1")
                _gather_raw(
                    nc.gpsimd,
                    out_ap=g1[:].rearrange("p (d c) -> p d c", c=E1),
                    in_ap=tab1[base:, 0:E1],
                    idxs_ap=idxg[:],
                    num_idxs=L, elem_size=E1, elem_step=R1,
                    queue_num=g % NQ)
                g1v = g1[:].rearrange("p (d c) -> p d c", c=E1)
                g1s = g1[:].bitcast(fp32).rearrange("p (d c) -> p d c", c=E1 // 2)

                sc = pb_sc.tile([P, D * 8], fp32, tag="scores")
                scv = sc[:].rearrange("p (d h) -> p d h", h=H1)
                strg_g = strgown_sb[:, g * 8:(g + 1) * 8]
                nc.vector.tensor_add(
                    out=scv, in0=g1s[:, :, 32:40],
                    in1=strg_g.rearrange("p (d h) -> p d h", d=1).to_broadcast([P, D, H1]))
                nc.vector.scalar_tensor_tensor(
                    out=sc[:], in0=sc[:], scalar=0.2, in1=sc[:],
                    op0=mybir.AluOpType.mult, op1=mybir.AluOpType.max)
                scb = pb_sc.tile([P, D * 8], bf16, tag="scb")
                nc.scalar.activation(out=scb[:], in_=sc[:], func=ACT.Exp)
                nc.vector.tensor_reduce(
                    out=ssumall[:, g * 8:(g + 1) * 8],
                    in_=scb[:].rearrange("p (d h) -> p h d", h=H1),
                    axis=mybir.AxisListType.X, op=mybir.AluOpType.add)

                msg = pb_msg.tile([P, D * HF], bf16, tag="msg")
                nc.vector.tensor_mul(
                    out=msg[:].rearrange("p (d h f) -> p d h f", h=H1, f=F1),
                    in0=g1v[:, :, 0:HF].rearrange("p d (h f) -> p d h f", f=F1),
                    in1=scb[:].rearrange("p (d h f) -> p d h f", h=H1, f=1
                                         ).to_broadcast([P, D, H1, F1]))
                nc.vector.tensor_reduce(
                    out=out1all[:, g * HF:(g + 1) * HF],
                    in_=msg[:].rearrange("p (d h f) -> p h f d", h=H1, f=F1),
                    axis=mybir.AxisListType.X, op=mybir.AluOpType.add)

            # ---- batched layer-1 tail + phase D, two halves for overlap ----
            def tail1(lo, hi):
                ng = hi - lo
                nc.vector.reciprocal(out=rinvall[:, lo * 8:hi * 8],
                                     in_=ssumall[:, lo * 8:hi * 8])
                o1 = out1all[:, lo * HF:hi * HF].rearrange(
                    "p (g h f) -> p g h f", h=H1, f=F1)
                nc.vector.tensor_mul(
                    out=o1, in0=o1,
                    in1=rinvall[:, lo * 8:hi * 8].rearrange(
                        "p (g h f) -> p g h f", h=H1, f=1
                    ).to_broadcast([P, ng, H1, F1]))
                # + b1, then ELU
                o = out1all[:, lo * HF:hi * HF]
                nc.vector.tensor_add(
                    out=o.rearrange("p (g c) -> p g c", c=HF),
                    in0=o.rearrange("p (g c) -> p g c", c=HF),
                    in1=b1bc_sb[:].rearrange("p (g c) -> p g c", g=1
                                             ).to_broadcast([P, ng, HF]))
                u = uuall[:, lo * HF:hi * HF]
                nc.vector.tensor_scalar_min(out=u, in0=o, scalar1=0.0)
                nc.scalar.activation(out=u, in_=u, func=ACT.Exp)
                nc.vector.tensor_scalar_max(out=o, in0=o, scalar1=0.0)
                nc.vector.tensor_add(out=o, in0=o, in1=u)
                nc.scalar.activation(out=o, in_=o, func=ACT.Copy, bias=-1.0)
                # transpose each group, project to layer-2 table rows
                for g in range(lo, hi):
                    psT = pd_ps.tile([HF, P], fp32, tag="psT")
                    nc.tensor.transpose(out=psT[:],
                                        in_=out1all[:, g * HF:(g + 1) * HF],
                                        identity=ident_sb[:])
                    hT = pd_ht.tile([HF, P], fp32, tag="hT")
                    nc.vector.tensor_copy(out=hT[:], in_=psT[:])
                    ps2 = pd_ps.tile([P, 9], fp32, tag="ps2")
                    nc.tensor.matmul(out=ps2[:], lhsT=hT[:],
                                     rhs=w2cat_sb[0:HF, 0:9],
                                     start=True, stop=True)
                    nc.vector.tensor_copy(out=ps2all[:, g * 9:(g + 1) * 9],
                                          in_=ps2[:])
                p2v = ps2all[:].rearrange("p (g c) -> p g c", c=9)
                nc.vector.tensor_add(
                    out=p2v[:, lo:hi, 0:8], in0=p2v[:, lo:hi, 0:8],
                    in1=b2bc_sb[:].rearrange("p (g c) -> p g c", g=1
                                             ).to_broadcast([P, hi - lo, 8]))
                nc.sync.dma_start(
                    out=tab2in.ap()[lo * P:hi * P, :].rearrange("(g p) c -> p g c", p=P),
                    in_=p2v[:, lo:hi, 0:8])

            # tail slices; first AllGather fires while late L1 groups gather
            q3 = (gmid // 3) or 1
            tail1(0, q3)
            tail1(q3, 2 * q3)
            tail1(2 * q3, gmid)
            nc.gpsimd.collective_compute(
                "AllGather",
                mybir.AluOpType.bypass,
                ins=[tab2in[0:nsA]],
                outs=[tab2cA[:]],
                replica_groups=[list(range(NC))],
            )
            nc.sync.dma_start(
                out=tab2f.ap()[0:NC * nslice, 0:8].rearrange("(c r) k -> c r k", r=nslice)[:, 0:nsA, :],
                in_=tab2cA.ap().rearrange("(c r) k -> c r k", r=nsA))
            tail1(gmid, g_cnt)
            nc.gpsimd.collective_compute(
                "AllGather",
                mybir.AluOpType.bypass,
                ins=[tab2in[nsA:nslice]],
                outs=[tab2cB[:]],
                replica_groups=[list(range(NC))],
            )
            nc.sync.dma_start(
                out=tab2f.ap()[0:NC * nslice, 0:8].rearrange("(c r) k -> c r k", r=nslice)[:, nsA:nslice, :],
                in_=tab2cB.ap().rearrange("(c r) k -> c r k", r=nsB))
            dummy2 = pp.tile([1, 8], fp32, tag="dummy2")
            nc.vector.memset(dummy2[:], 0.0)
            nc.vector.memset(dummy2[:, 7:8], MASKVAL)
            nc.sync.dma_start(out=tab2f.ap()[npad:npad + 1, 0:8], in_=dummy2[:])

            # ---- phase E: layer 2 per group (big ops only) ----
            for g in range(g_cnt):
                D = dg[g]
                L = P * D
                idxg = pidx.tile([P, (offs[g + 1] - offs[g]) // 16], DT.int16, tag="idxg")
                nc.sync.dma_start(out=idxg[:], in_=idx[:, offs[g] // 16:offs[g + 1] // 16])
                g2 = pe_g.tile([P, D * E2], fp32, tag="g2")
                _gather_raw(
                    nc.gpsimd,
                    out_ap=g2[:].rearrange("p (d c) -> p d c", c=E2),
                    in_ap=tab2f[base:, 0:E2],
                    idxs_ap=idxg[:],
                    num_idxs=L, elem_size=E2, elem_step=R2,
                    queue_num=g % NQ)
                g2v = g2[:].rearrange("p (d c) -> p d c", c=E2)

                sc2 = pe_sc.tile([P, D], fp32, tag="sc2")
                nc.vector.tensor_scalar_add(
                    out=sc2[:],
                    in0=g2v[:, :, 7:8].rearrange("p d c -> p (d c)"),
                    scalar1=ps2all[:, g * 9 + 8:g * 9 + 9])
                nc.vector.scalar_tensor_tensor(
                    out=sc2[:], in0=sc2[:], scalar=0.2, in1=sc2[:],
                    op0=mybir.AluOpType.mult, op1=mybir.AluOpType.max)
                nc.scalar.activation(out=sc2[:], in_=sc2[:], func=ACT.Exp,
                                     accum_out=ssum2all[:, g:g + 1])
                m2 = pm2.tile([P, D * 8], fp32, tag="m2")
                nc.vector.tensor_mul(
                    out=m2[:].rearrange("p (d c) -> p d c", c=8),
                    in0=g2v[:, :, 0:8],
                    in1=sc2[:].rearrange("p (d c) -> p d c", c=1).to_broadcast([P, D, 8]))
                nc.vector.tensor_reduce(
                    out=o2all[:, g * C2:(g + 1) * C2],
                    in_=m2[:].rearrange("p (d c) -> p c d", c=8)[:, 0:C2, :],
                    axis=mybir.AxisListType.X, op=mybir.AluOpType.add)

            # ---- batched layer-2 tail: normalize + softmax + one DMA out ----
            nc.vector.reciprocal(out=rinv2all[:], in_=ssum2all[:])
            o2v = o2all[:].rearrange("p (g c) -> p g c", c=C2)
            nc.vector.tensor_mul(
                out=o2v, in0=o2v,
                in1=rinv2all[:].rearrange("p (g c) -> p g c", c=1
                                          ).to_broadcast([P, g_cnt, C2]))
            # softmax over the 7 logits (range-safe without max subtraction)
            nc.scalar.activation(out=e7all[:], in_=o2all[:], func=ACT.Exp)
            nc.vector.tensor_reduce(
                out=sum7all[:], in_=e7all[:].rearrange("p (g c) -> p g c", c=C2),
                axis=mybir.AxisListType.X, op=mybir.AluOpType.add)
            nc.vector.reciprocal(out=r7all[:], in_=sum7all[:])
            nc.vector.tensor_mul(
                out=resall[:].rearrange("p (g c) -> p g c", c=C2),
                in0=e7all[:].rearrange("p (g c) -> p g c", c=C2),
                in1=r7all[:].rearrange("p (g c) -> p g c", c=1
                                       ).to_broadcast([P, g_cnt, C2]))
            nc.sync.dma_start(
                out=out.ap().rearrange("(g p) c -> p g c", p=P),
                in_=resall[:].rearrange("p (g c) -> p g c", c=C2))

    nc.compile()
    return nc


# --------------------------------------------------------------------------
# host side
# --------------------------------------------------------------------------

def _preprocess(x, edge_index):
    src = np.asarray(edge_index[0], np.int64)
    trg = np.asarray(edge_index[1], np.int64)
    n = x.shape[0]
    e = src.shape[0]

    deg = np.bincount(trg, minlength=n)
    order = np.argsort(-deg, kind="stable")          # rank -> node
    g_cnt = math.ceil(n / (P * NC))
    if g_cnt * P * NC == n:
        g_cnt += 1  # ensure pad rows exist (dummy index must be a pad row)
    npad = g_cnt * P * NC
    nslice = g_cnt * P

    ranks = np.empty(n, np.int64)
    ranks[order] = np.arange(n)
    core_of = ranks % NC
    grp_of = ranks // (P * NC)
    slot_of = (ranks // NC) % P
    perm = core_of * nslice + grp_of * P + slot_of   # node -> perm position

    dg = []
    for g in range(g_cnt):
        w = order[P * NC * g: P * NC * (g + 1)]
        if len(w) == 0:
            dg.append(1)
            continue
        degs = deg[w]  # already descending
        dmax = max(int(degs.max()), 1)
        if len(degs) <= 1016 or int(degs[1016:].max()) == dmax:
            dmax += 1
        dg.append(dmax)
    offs = np.concatenate([[0], np.cumsum([P * d for d in dg])]).astype(np.int64)
    total_slots = int(offs[-1])

    dummy = npad  # the dedicated poison row appended to both tables
    base = 32768 if npad > 32768 else 0

    tp = perm[trg]
    eorder = np.argsort(tp, kind="stable")
    tps = tp[eorder]
    counts = np.bincount(tps, minlength=npad)
    starts = np.concatenate([[0], np.cumsum(counts)[:-1]])
    d_of = np.arange(e) - starts[tps]

    c_of = tps // nslice
    r_local = tps % nslice
    g_of = r_local // P
    p_of = r_local % P
    pos = offs[g_of] + d_of * P + p_of               # k = d*128 + p within group

    idx_flat = np.full((NC, total_slots), dummy - base, np.int16)
    idx_flat[c_of, pos] = (perm[src[eorder]] - base).astype(np.int16)

    idx_wrapped = np.empty((NC, P, total_slots // 16), np.int16)
    for g in range(g_cnt):
        lo, hi = int(offs[g]), int(offs[g + 1])
        blk = idx_flat[:, lo:hi].reshape(NC, (hi - lo) // 16, 16)  # [c, col, p16]
        wr = np.swapaxes(blk, 1, 2)                                # [c, p16, col]
        idx_wrapped[:, :, lo // 16:hi // 16] = np.tile(wr, (1, 8, 1))

    pad_mask = np.ones(npad, bool)
    pad_mask[perm] = False

    return dict(n=n, e=e, npad=npad, nslice=nslice, g_cnt=g_cnt, dg=dg,
                total_slots=total_slots, perm=perm, idx_wrapped=idx_wrapped,
                pad_mask=pad_mask, base=base)


def _prepare(x, edge_index, W1, a_src1, a_trg1, b1, W2, a_src2, a_trg2, b2):
    x = np.asarray(x, np.float32)
    W1 = np.asarray(W1, np.float32)
    a_src1 = np.asarray(a_src1, np.float32)
    a_trg1 = np.asarray(a_trg1, np.float32)
    b1 = np.asarray(b1, np.float32)
    W2 = np.asarray(W2, np.float32)
    a_src2 = np.asarray(a_src2, np.float32)
    a_trg2 = np.asarray(a_trg2, np.float32)
    b2 = np.asarray(b2, np.float32)

    meta = _preprocess(x, edge_index)
    npad, nslice, g_cnt = meta["npad"], meta["nslice"], meta["g_cnt"]
    perm = meta["perm"]

    xp = np.zeros((npad, FIN), np.float32)
    xp[perm] = x
    xT = np.ascontiguousarray(xp.T).astype(BF16)

    Wt = W1.T  # [128, 64], col = h*F + j
    w3 = W1.reshape(H1, F1, FIN)
    Asrc = np.einsum("hjf,hj->fh", w3, a_src1[0])
    Atrg = np.einsum("hjf,hj->fh", w3, a_trg1[0])
    wcat1 = np.concatenate([Wt, Asrc, Atrg], axis=1).astype(BF16)

    w2cat = np.zeros((HF, 16), np.float32)
    w2cat[:, 0:C2] = W2.T
    w2cat[:, C2] = W2.T @ a_src2[0, 0]
    w2cat[:, C2 + 1] = W2.T @ a_trg2[0, 0]
    w2cat = np.vstack([w2cat, w2cat])

    key = (npad, g_cnt, tuple(meta["dg"]))
    if key not in _CACHE:
        _CACHE[key] = _build(npad, nslice, g_cnt, meta["dg"], meta["total_slots"])
    nc = _CACHE[key]

    in_maps = []
    for c in range(NC):
        xownT = np.ascontiguousarray(xp[c * nslice:(c + 1) * nslice].T).astype(BF16)
        in_maps.append({
            "xT": xT,
            "xownT": xownT,
            "idx": np.ascontiguousarray(meta["idx_wrapped"][c]),
            "wcat1": wcat1,
            "w2cat": w2cat,
            "b1d": b1,
            "b2d": b2,
        })
    return nc, in_maps, meta


def kernel(x, edge_index, W1, a_src1, a_trg1, b1, W2, a_src2, a_trg2, b2):
    nc, in_maps, meta = _prepare(x, edge_index, W1, a_src1, a_trg1, b1,
                                 W2, a_src2, a_trg2, b2)
    res = run_bass_kernel_spmd(nc, in_maps, core_ids=list(range(NC)))
    full = np.concatenate([res.results[c]["out"] for c in range(NC)], axis=0)
    return full[meta["perm"]].astype(np.float32)
